# revision 28
# baseline (speedup 1.0000x reference)
"""Causal self-attention (B=2, T=2048, dim=2048, H=16, D=128) on 8 trn2 NeuronCores.

Sharding: data-parallel over batch (2 groups of 4 cores), tensor-parallel over
heads within a group (4 heads/core).  Each core computes its heads' QKV
projection (x @ Wqkv_part^T), RoPE, causal attention, and a partial output
projection against its W_proj column block; the host sums the 4 partials per
batch element.

v8 schedule (all matmul operands bf16, fp32 accumulation): one readiness-aware
software pipeline — attention units of window w are woven between the QKV
matmul sets of the SAME window and the proj units of window w-1, so the
ScalarE exp stream always drains under QKV/proj PE work:
  - all DRAM operands host-packed so every DMA lands in >=4KB contiguous
    per-partition lines; startup loads split across the two HWDGE queues
    (sync + scalar) in first-use order; first weight slab split so the
    first matmul gates on 128KB; rope tables shipped as 64 rows and
    mirrored on-chip (rows 0-63 == rows 64-127 up to the rotate sign).
  - RoPE fused into the QKV PSUM evacuation on DVE: rot halves are
    partition-offset multiplies against the sign-folded sin table, then
    dst = ps*cos + rot.  No PE swap matmul, no separate evacuation cast.
  - scores pipelined depth-3: PE score matmul -> ScalarE exp (bf16) -> PE
    PV.  Each head-window's pt tiles chain-accumulate into one fp16 tile
    on DVE (one add per unit), so the PE rowsum is a single 512-col
    all-ones matmul per head (denominators replicated across partitions).
  - softmax normalization off the PE: reciprocal_approx_fast + multiply (DVE).
  - QKV/proj PSUM evacuation on DVE/ScalarE; y DMA'd per window.
"""

import os

import numpy as np
import ml_dtypes

import concourse.bass as bass
import concourse.bacc as bacc
import concourse.tile as tile
import concourse.mybir as mybir
from concourse import bass_utils

BF16 = mybir.dt.bfloat16
F16 = mybir.dt.float16
F32 = mybir.dt.float32

B, T, DIM = 2, 2048, 2048
H, D = 16, 128
HL = 4                   # heads per core
NCORES = 8
NCHUNK = DIM // 128      # 16 contraction chunks
NW = T // 512            # 4 query windows
NTT = T // 128           # 16 token tiles
SCALE = 1.0 / float(np.sqrt(D))

_CACHE = {}
LAST_RESULTS = None


def _build_module():
    nc = bacc.Bacc("TRN2", target_bir_lowering=False, debug=False)
    # host-packed layouts: partition dim first, contiguous per-partition lines
    xPd = nc.dram_tensor("xP", (128, NW, NCHUNK, 512), BF16, kind="ExternalInput")
    wqPd = nc.dram_tensor("wqP", (128, 8, NCHUNK, 128), BF16, kind="ExternalInput")
    wvPd = nc.dram_tensor("wvP", (128, NCHUNK, 512), BF16, kind="ExternalInput")
    wpPd = nc.dram_tensor("wpP", (128, HL, DIM), BF16, kind="ExternalInput")
    cosT = nc.dram_tensor("cosT", (64, T), BF16, kind="ExternalInput")
    sinT = nc.dram_tensor("sinT", (64, T), BF16, kind="ExternalInput")
    tri = nc.dram_tensor("tri", (128, 128), BF16, kind="ExternalInput")
    ones = nc.dram_tensor("ones", (128, 128), F16, kind="ExternalInput")
    y = nc.dram_tensor("y", (T, DIM), F32, kind="ExternalOutput")

    Exp = mybir.ActivationFunctionType.Exp

    with tile.TileContext(nc) as tc:
        with (
            tc.tile_pool(name="const", bufs=1) as cpool,
            tc.tile_pool(name="xp", bufs=2) as xpool,
            tc.tile_pool(name="rotp", bufs=3) as rotpool,
            tc.tile_pool(name="ptp", bufs=6) as ptpool,
            tc.tile_pool(name="rsap", bufs=2) as rsapool,
            tc.tile_pool(name="rcpp", bufs=2) as rcppool,
            tc.tile_pool(name="yp", bufs=3) as ypool,
            tc.tile_pool(name="psA", bufs=2, space="PSUM") as psA,
            tc.tile_pool(name="psS", bufs=3, space="PSUM") as psS,
            tc.tile_pool(name="psO", bufs=2, space="PSUM") as psO,
            tc.tile_pool(name="psR", bufs=1, space="PSUM") as psR,
        ):
            # per-set weight slabs: dependency granularity is per-tile, so a
            # qk set only waits for its own slab's DMA; slab 0 split so the
            # first matmul gates on a 128KB transfer
            wq0a = cpool.tile([128, 2, 128], BF16, tag="wq0a", name="wq0a")
            wq0b = cpool.tile([128, 14, 128], BF16, tag="wq0b", name="wq0b")
            wq_slab = [None] + [
                cpool.tile([128, NCHUNK, 128], BF16, tag=f"wq{s}", name=f"wq{s}")
                for s in range(1, 8)
            ]
            wv_sb = cpool.tile([128, NCHUNK, 512], BF16, tag="wv")
            wp_sb = cpool.tile([128, HL, DIM], BF16, tag="wp")
            cos_sb = cpool.tile([128, T], BF16, tag="cos")
            sin_sb = cpool.tile([128, T], BF16, tag="sin")
            tri_sb = cpool.tile([128, 128], BF16, tag="tri")
            ones_sb = cpool.tile([128, 128], F16, tag="ones")
            q_sb = cpool.tile([128, HL * T], BF16, tag="q")
            k_sb = cpool.tile([128, HL * T], BF16, tag="k")
            v_sb = cpool.tile([128, NTT * HL * D], BF16, tag="v")
            o_sb = cpool.tile([128, HL * T], BF16, tag="o")

            xbs = {}

            def load_xb(w):
                xb = xpool.tile([128, NCHUNK, 512], BF16, tag="xb", name="xb")
                nc.sync.dma_start(xb[:], xPd[:, w])
                xbs[w] = xb

            # Startup DMAs, first-needed first, split across both HWDGE
            # queues (sync + scalar); xb0 in 2-chunk pieces alternating
            # queues so set-0's chunk loop rarely outruns arrival; half-height
            # rope tables on the gpsimd SWDGE, mirrored on-chip by DVE.
            xb0 = xpool.tile([128, NCHUNK, 512], BF16, tag="xb", name="xb0")
            xbs[0] = xb0
            nc.scalar.dma_start(wq0a[:], wqPd[:, 0, 0:2, :])
            nc.sync.dma_start(xb0[:, 0:1, :], xPd[:, 0, 0:1, :])
            nc.gpsimd.dma_start(cos_sb[0:64, :], cosT[:, :])
            nc.gpsimd.dma_start(sin_sb[64:128, :], sinT[:, :])
            nc.scalar.dma_start(wq0b[:], wqPd[:, 0, 2:16, :])
            nc.sync.dma_start(xb0[:, 1:3, :], xPd[:, 0, 1:3, :])
            nc.sync.dma_start(xb0[:, 3:6, :], xPd[:, 0, 3:6, :])
            nc.scalar.dma_start(xb0[:, 6:8, :], xPd[:, 0, 6:8, :])
            nc.sync.dma_start(xb0[:, 8:10, :], xPd[:, 0, 8:10, :])
            nc.scalar.dma_start(wq_slab[1][:], wqPd[:, 1])
            nc.sync.dma_start(xb0[:, 10:12, :], xPd[:, 0, 10:12, :])
            nc.scalar.dma_start(xb0[:, 12:14, :], xPd[:, 0, 12:14, :])
            nc.sync.dma_start(xb0[:, 14:16, :], xPd[:, 0, 14:16, :])
            nc.gpsimd.dma_start(tri_sb[:], tri[:, :])
            nc.gpsimd.dma_start(ones_sb[:], ones[:, :])
            nc.scalar.dma_start(wq_slab[2][:], wqPd[:, 2])
            nc.sync.dma_start(wq_slab[3][:], wqPd[:, 3])
            nc.sync.dma_start(wv_sb[:, 0:8, :], wvPd[:, 0:8, :])
            nc.scalar.dma_start(wv_sb[:, 8:16, :], wvPd[:, 8:16, :])
            nc.sync.dma_start(wq_slab[4][:], wqPd[:, 4])
            nc.scalar.dma_start(wq_slab[5][:], wqPd[:, 5])
            nc.sync.dma_start(wq_slab[6][:], wqPd[:, 6])
            nc.scalar.dma_start(wq_slab[7][:], wqPd[:, 7])
            # mirror the half-height tables on DVE (cos rows repeat; sin rows
            # 0-63 carry the folded rotate-half sign)
            nc.vector.tensor_copy(cos_sb[64:128, :], cos_sb[0:64, :])
            nc.vector.tensor_scalar_mul(sin_sb[0:64, :], sin_sb[64:128, :], -1.0)

            def qkv_bigs(w):
                """[('q'|'k'|'v', idx, closure)] — 8 qk sets then 4 v sets."""
                wsl = slice(w * 512, (w + 1) * 512)

                def qk_set(grp, j):
                    def run():
                        xb = xbs[w]
                        dst = q_sb if grp == 0 else k_sb
                        ps = psA.tile([128, 512], F32, tag="ps", name="ps")
                        s = grp * 4 + j
                        for c in range(NCHUNK):
                            if s == 0:
                                slab = wq0a[:, c, :] if c < 2 else wq0b[:, c - 2, :]
                            else:
                                slab = wq_slab[s][:, c, :]
                            nc.tensor.matmul(
                                ps[:],
                                slab,
                                xb[:, c, :],
                                start=(c == 0),
                                stop=(c == NCHUNK - 1),
                            )
                        # fused RoPE evacuation: dst = ps*cos + rot(ps)*sin
                        sl = slice(j * T + w * 512, j * T + (w + 1) * 512)
                        rot = rotpool.tile([128, 512], BF16, tag="rot", name="rot")
                        nc.vector.tensor_mul(
                            rot[0:64, :], ps[64:128, :], sin_sb[0:64, wsl]
                        )
                        nc.vector.tensor_mul(
                            rot[64:128, :], ps[0:64, :], sin_sb[64:128, wsl]
                        )
                        nc.vector.tensor_mul(dst[:, sl], ps[:], cos_sb[:, wsl])
                        nc.vector.tensor_add(dst[:, sl], dst[:, sl], rot[:])

                    return run

                def v_set(ttl):
                    def run():
                        xb = xbs[w]
                        ttg = w * 4 + ttl
                        ps = psA.tile([128, 512], F32, tag="ps", name="ps")
                        for c in range(NCHUNK):
                            nc.tensor.matmul(
                                ps[:],
                                xb[:, c, ttl * 128 : (ttl + 1) * 128],
                                wv_sb[:, c, :],
                                start=(c == 0),
                                stop=(c == NCHUNK - 1),
                            )
                        nc.scalar.copy(v_sb[:, ttg * 512 : (ttg + 1) * 512], ps[:])

                    return run

                return (
                    [("q", j, qk_set(0, j)) for j in range(HL)]
                    + [("v", ttl, v_set(ttl)) for ttl in range(4)]
                    + [("k", j, qk_set(1, j)) for j in range(HL)]
                )

            def attn_fillers(w, pos):
                """Per-(head, key-tile) closures + their readiness (number of
                this window's qkv bigs that must have been emitted first).
                All of a head-window's pt tiles chain-accumulate into one
                fp16 acc on DVE (one add per unit); a single 512-col
                all-ones matmul per head produces the softmax denominators,
                emitted one unit into the next head so the single-bank
                rowsum tile's WAR on the previous reciprocal is covered."""
                nkt = 4 * w + 4
                OD = 4 * w            # off-diagonal tiles per head
                nu = HL * nkt
                state = {
                    "pend": [],
                    "issued": 0,
                    "oT": {},
                    "pt0": None,
                    "acc": {},
                }

                def geom(kt):
                    if kt < OD:
                        return 512 * w, 512, False
                    q0 = 128 * kt
                    return q0, 512 * (w + 1) - 128 * kt, True

                def issue_score(u):
                    h, kt = divmod(u, nkt)
                    hq = h * T
                    q0, n, diag = geom(kt)
                    st = psS.tile([128, 512], F32, tag="st", name="st")
                    nc.tensor.matmul(
                        st[:, :n],
                        k_sb[:, hq + kt * 128 : hq + (kt + 1) * 128],
                        q_sb[:, hq + q0 : hq + q0 + n],
                        start=True,
                        stop=True,
                    )
                    pt = ptpool.tile([128, 512], BF16, tag="pt", name="pt")
                    nc.scalar.activation(
                        pt[:, :n], st[:, :n], Exp, bias=0.0, scale=SCALE
                    )
                    if diag:
                        nc.vector.tensor_mul(pt[:, 0:128], pt[:, 0:128], tri_sb[:])
                    return pt, q0, n

                DEPTH = 3

                def flush_rs(h):
                    # single rowsum matmul for head h, then the off-PE
                    # normalization chain
                    rs = psR.tile([128, 512], F32, tag="rs", name="rs")
                    nc.tensor.matmul(
                        rs[:],
                        ones_sb[:],
                        state["acc"].pop(h)[:],
                        start=True,
                        stop=True,
                    )
                    hq = h * T
                    rcp = rcppool.tile([128, 512], F32, tag="rcp", name="rcp")
                    nc.vector.reciprocal_approx_fast(rcp[:], rs[:])
                    nc.vector.tensor_mul(
                        o_sb[:, hq + w * 512 : hq + (w + 1) * 512],
                        state["oT"].pop(h)[:],
                        rcp[:],
                    )

                def unit(u):
                    def run():
                        h, kt = divmod(u, nkt)
                        while state["issued"] < min(u + DEPTH + 1, nu):
                            state["pend"].append(issue_score(state["issued"]))
                            state["issued"] += 1
                        if kt == 0:
                            state["oT"][h] = psO.tile(
                                [128, 512], F32, tag="oT", name="oT"
                            )
                        pt, q0, n = state["pend"].pop(0)
                        off = q0 - 512 * w
                        nc.tensor.matmul(
                            state["oT"][h][:, off:512],
                            v_sb[:, kt * 512 + h * 128 : kt * 512 + (h + 1) * 128],
                            pt[:, :n],
                            start=(kt == 0),
                            stop=(kt == nkt - 1),
                        )
                        # denominator accumulation (kt = 0 always has n = 512)
                        if kt == 0:
                            state["pt0"] = pt
                        elif kt == 1:
                            acc = rsapool.tile(
                                [128, 512], F16, tag="acc", name="acc"
                            )
                            if n == 512:
                                nc.vector.tensor_add(
                                    acc[:], state["pt0"][:], pt[:]
                                )
                            else:
                                nc.vector.tensor_copy(
                                    acc[:, 0:off], state["pt0"][:, 0:off]
                                )
                                nc.vector.tensor_add(
                                    acc[:, off:512],
                                    state["pt0"][:, off:512],
                                    pt[:, :n],
                                )
                            state["acc"][h] = acc
                        else:
                            acc = state["acc"][h]
                            nc.vector.tensor_add(
                                acc[:, off:512], acc[:, off:512], pt[:, :n]
                            )
                        if kt == 1 and h > 0:
                            flush_rs(h - 1)
                        if u == nu - 1:
                            flush_rs(h)

                    return run

                def score_ra(u):
                    h, kt = divmod(u, nkt)
                    if kt < OD:
                        return pos[("q", h)] + 1
                    return pos[("k", h)] + 1

                def pv_ra(u):
                    h, kt = divmod(u, nkt)
                    if kt < OD:
                        return 0
                    return pos[("v", kt - OD)] + 1

                fillers = []
                ready = []
                for u in range(nu):
                    ra = pv_ra(u)
                    for uu in range(u, min(u + DEPTH + 1, nu)):
                        ra = max(ra, score_ra(uu))
                    fillers.append(unit(u))
                    ready.append(ra)
                return fillers, ready

            def proj_bigs(w, pools=None, alt_dma=False):
                out = []

                def unit(tt, nwi, pool, ptag, dve_evac, dmae):
                    def run():
                        yps = pool.tile([128, 512], F32, tag=ptag, name="yps")
                        for hh in range(HL):
                            nc.tensor.matmul(
                                yps[:],
                                o_sb[:, hh * T + tt * 128 : hh * T + (tt + 1) * 128],
                                wp_sb[:, hh, nwi * 512 : (nwi + 1) * 512],
                                start=(hh == 0),
                                stop=(hh == HL - 1),
                            )
                        ysb = ypool.tile([128, 512], F32, tag="ysb", name="ysb")
                        if dve_evac:
                            nc.vector.tensor_copy(ysb[:], yps[:])
                        else:
                            nc.scalar.copy(ysb[:], yps[:])
                        dmae.dma_start(
                            y[tt * 128 : (tt + 1) * 128, nwi * 512 : (nwi + 1) * 512],
                            ysb[:],
                        )

                    return run

                i = 0
                for tt in range(4 * w, 4 * w + 4):
                    for nwi in range(DIM // 512):
                        if pools:
                            pool, ptag = pools[i % len(pools)]
                        else:
                            pool, ptag = psA, "ps"
                        # in the final batch (attention over) split the
                        # evacuations across ACT and DVE and put each unit's
                        # DMA config on the opposite sequencer so neither
                        # sequencer serializes copy + trigger
                        dve = alt_dma and (i % 2 == 1)
                        dmae = nc.scalar if dve else nc.sync
                        out.append(
                            ("p", i, unit(tt, nwi, pool, ptag, dve, dmae))
                        )
                        i += 1
                return out

            def weave(bigs, fillers, ready):
                nb = len(bigs)
                nf = len(fillers)
                done = 0
                for i, (_, _, b) in enumerate(bigs):
                    b()
                    # cap the per-big burst so late-ready units don't flood
                    # the exp engine all at once
                    target = min(int(round(nf * (i + 1) / nb)), done + 4)
                    while done < nf and done < target and ready[done] <= i + 1:
                        fillers[done]()
                        done += 1
                while done < nf:
                    fillers[done]()
                    done += 1

            for w in range(NW):
                bigs = qkv_bigs(w)
                if w + 1 < NW:
                    # prefetch next window's x once the startup crunch is over
                    bigs.insert(6, ("x", w + 1, lambda wn=w + 1: load_xb(wn)))
                if w == 0:
                    bigs.append(
                        ("wp", 0, lambda: nc.sync.dma_start(wp_sb[:], wpPd[:]))
                    )
                if w >= 1:
                    bigs += proj_bigs(w - 1)
                pos = {(lbl, idx): i for i, (lbl, idx, _) in enumerate(bigs)}
                fillers, ready = attn_fillers(w, pos)
                weave(bigs, fillers, ready)
            # after the last window's attention, all PSUM pools are free:
            # cycle the final proj units across them so the evacuation chain
            # never blocks the PE on a bank WAR
            last = proj_bigs(NW - 1, alt_dma=True)
            for _, _, fn in last[:-1]:
                fn()
            # final unit split into column halves on both HWDGE queues so the
            # exposed tail evacuation + y DMA is halved
            tt, nwi = NTT - 1, DIM // 512 - 1
            yps = psA.tile([128, 512], F32, tag="ps", name="yps")
            for hh in range(HL):
                nc.tensor.matmul(
                    yps[:],
                    o_sb[:, hh * T + tt * 128 : hh * T + (tt + 1) * 128],
                    wp_sb[:, hh, nwi * 512 : (nwi + 1) * 512],
                    start=(hh == 0),
                    stop=(hh == HL - 1),
                )
            ysb = ypool.tile([128, 512], F32, tag="ysb", name="ysb")
            r0 = slice(tt * 128, (tt + 1) * 128)
            nc.scalar.copy(ysb[:, 0:256], yps[:, 0:256])
            nc.sync.dma_start(
                y[r0, nwi * 512 : nwi * 512 + 256], ysb[:, 0:256]
            )
            nc.scalar.copy(ysb[:, 256:512], yps[:, 256:512])
            nc.scalar.dma_start(
                y[r0, nwi * 512 + 256 : (nwi + 1) * 512], ysb[:, 256:512]
            )

    nc.compile()
    return nc


def _rope_tables():
    inv_freq = (
        1.0 / (10000.0 ** (np.arange(0, D, 2, dtype=np.float32) / np.float32(D)))
    ).astype(np.float32)
    tpos = np.arange(T, dtype=np.float32)
    freqs = tpos[:, None] * inv_freq[None, :]  # (T, 64)
    cosT = np.ascontiguousarray(np.cos(freqs).T)  # (64, T)
    sinT = np.ascontiguousarray(np.sin(freqs).T)
    return (
        cosT.astype(ml_dtypes.bfloat16),
        sinT.astype(ml_dtypes.bfloat16),
    )


def make_in_maps(x, W_qkv, W_proj):
    cosT, sinT = _rope_tables()
    tri = (np.arange(128)[None, :] >= np.arange(128)[:, None]).astype(
        ml_dtypes.bfloat16
    )
    tri = np.ascontiguousarray(tri)
    ones = np.ones((128, 128), dtype=np.float16)

    xPs = {}
    for b in range(B):
        xt = np.ascontiguousarray(x[b].T).astype(ml_dtypes.bfloat16)  # (dim, T)
        xPs[b] = np.ascontiguousarray(
            xt.reshape(NCHUNK, 128, NW, 512).transpose(1, 2, 0, 3)
        )

    wqPs, wvPs, wpPs = {}, {}, {}
    for g in range(4):
        Wq = W_qkv[512 * g : 512 * (g + 1)]
        Wk = W_qkv[2048 + 512 * g : 2048 + 512 * (g + 1)]
        Wv = W_qkv[4096 + 512 * g : 4096 + 512 * (g + 1)]
        Wc = np.concatenate([Wq, Wk, Wv], axis=0)  # (1536, 2048)
        A = (
            np.ascontiguousarray(Wc.T)
            .astype(ml_dtypes.bfloat16)
            .reshape(NCHUNK, 128, 1536)
            .transpose(1, 0, 2)
        )  # [p, c, e]
        wqPs[g] = np.ascontiguousarray(
            A[:, :, :1024].reshape(128, NCHUNK, 8, 128).transpose(0, 2, 1, 3)
        )  # [p, s, c, e]
        wvPs[g] = np.ascontiguousarray(A[:, :, 1024:])  # [p, c, e512]
        wpPs[g] = np.ascontiguousarray(
            np.ascontiguousarray(W_proj[:, 512 * g : 512 * (g + 1)].T)
            .astype(ml_dtypes.bfloat16)
            .reshape(HL, 128, DIM)
            .transpose(1, 0, 2)
        )  # [p, h, n]

    in_maps = []
    for c in range(NCORES):
        b, g = divmod(c, 4)
        in_maps.append(
            {
                "xP": xPs[b],
                "wqP": wqPs[g],
                "wvP": wvPs[g],
                "wpP": wpPs[g],
                "cosT": cosT,
                "sinT": sinT,
                "tri": tri,
                "ones": ones,
            }
        )
    return in_maps


def kernel(x, W_qkv, W_proj):
    global LAST_RESULTS
    x = np.asarray(x, dtype=np.float32)
    W_qkv = np.asarray(W_qkv, dtype=np.float32)
    W_proj = np.asarray(W_proj, dtype=np.float32)
    assert x.shape == (B, T, DIM) and W_qkv.shape == (3 * H * D, DIM)

    if "nc" not in _CACHE:
        _CACHE["nc"] = _build_module()
    nc = _CACHE["nc"]

    in_maps = make_in_maps(x, W_qkv, W_proj)
    trace = os.environ.get("KERNEL_TRACE", "0") == "1"
    res = bass_utils.run_bass_kernel_spmd(
        nc, in_maps, core_ids=list(range(NCORES)), trace=trace
    )
    LAST_RESULTS = res
    y = np.zeros((B, T, DIM), dtype=np.float32)
    for c in range(NCORES):
        y[c // 4] += res.results[c]["y"]
    return y


# revision 29
# speedup vs baseline: 1.0092x; 1.0092x over previous
"""Causal self-attention (B=2, T=2048, dim=2048, H=16, D=128) on 8 trn2 NeuronCores.

Sharding: data-parallel over batch (2 groups of 4 cores), tensor-parallel over
heads within a group (4 heads/core).  Each core computes its heads' QKV
projection (x @ Wqkv_part^T), RoPE, causal attention, and a partial output
projection against its W_proj column block; the host sums the 4 partials per
batch element.

v8 schedule (all matmul operands bf16, fp32 accumulation): one readiness-aware
software pipeline — attention units of window w are woven between the QKV
matmul sets of the SAME window and the proj units of window w-1, so the
ScalarE exp stream always drains under QKV/proj PE work:
  - all DRAM operands host-packed so every DMA lands in >=4KB contiguous
    per-partition lines; startup loads split across the two HWDGE queues
    (sync + scalar) in first-use order; first weight slab split so the
    first matmul gates on 128KB; rope tables shipped as 64 rows and
    mirrored on-chip (rows 0-63 == rows 64-127 up to the rotate sign).
  - RoPE fused into the QKV PSUM evacuation on DVE: rot halves are
    partition-offset multiplies against the sign-folded sin table, then
    dst = ps*cos + rot.  No PE swap matmul, no separate evacuation cast.
  - scores pipelined depth-3: PE score matmul -> ScalarE exp (bf16) -> PE
    PV.  Each head-window's pt tiles chain-accumulate into one fp16 tile
    on DVE (one add per unit), so the PE rowsum is a single 512-col
    all-ones matmul per head (denominators replicated across partitions).
  - softmax normalization off the PE: reciprocal_approx_fast + multiply (DVE).
  - QKV/proj PSUM evacuation on DVE/ScalarE; y DMA'd per window.
"""

import os

import numpy as np
import ml_dtypes

import concourse.bass as bass
import concourse.bacc as bacc
import concourse.tile as tile
import concourse.mybir as mybir
from concourse import bass_utils

BF16 = mybir.dt.bfloat16
F16 = mybir.dt.float16
F32 = mybir.dt.float32

B, T, DIM = 2, 2048, 2048
H, D = 16, 128
HL = 4                   # heads per core
NCORES = 8
NCHUNK = DIM // 128      # 16 contraction chunks
NW = T // 512            # 4 query windows
NTT = T // 128           # 16 token tiles
SCALE = 1.0 / float(np.sqrt(D))

_CACHE = {}
LAST_RESULTS = None


def _build_module():
    nc = bacc.Bacc("TRN2", target_bir_lowering=False, debug=False)
    # host-packed layouts: partition dim first, contiguous per-partition lines
    xPd = nc.dram_tensor("xP", (128, NW, NCHUNK, 512), BF16, kind="ExternalInput")
    wqPd = nc.dram_tensor("wqP", (128, 8, NCHUNK, 128), BF16, kind="ExternalInput")
    wvPd = nc.dram_tensor("wvP", (128, NCHUNK, 512), BF16, kind="ExternalInput")
    wpPd = nc.dram_tensor("wpP", (128, HL, DIM), BF16, kind="ExternalInput")
    cosT = nc.dram_tensor("cosT", (64, T), BF16, kind="ExternalInput")
    sinT = nc.dram_tensor("sinT", (64, T), BF16, kind="ExternalInput")
    tri = nc.dram_tensor("tri", (128, 128), BF16, kind="ExternalInput")
    ones = nc.dram_tensor("ones", (128, 128), F16, kind="ExternalInput")
    y = nc.dram_tensor("y", (T, DIM), F32, kind="ExternalOutput")

    Exp = mybir.ActivationFunctionType.Exp

    with tile.TileContext(nc) as tc:
        with (
            tc.tile_pool(name="const", bufs=1) as cpool,
            tc.tile_pool(name="xp", bufs=2) as xpool,
            tc.tile_pool(name="rotp", bufs=3) as rotpool,
            tc.tile_pool(name="ptp", bufs=6) as ptpool,
            tc.tile_pool(name="rsap", bufs=2) as rsapool,
            tc.tile_pool(name="rcpp", bufs=2) as rcppool,
            tc.tile_pool(name="yp", bufs=3) as ypool,
            tc.tile_pool(name="psA", bufs=2, space="PSUM") as psA,
            tc.tile_pool(name="psS", bufs=3, space="PSUM") as psS,
            tc.tile_pool(name="psO", bufs=2, space="PSUM") as psO,
            tc.tile_pool(name="psR", bufs=1, space="PSUM") as psR,
        ):
            # per-set weight slabs: dependency granularity is per-tile, so a
            # qk set only waits for its own slab's DMA; slab 0 split so the
            # first matmul gates on a 128KB transfer
            wq0a = cpool.tile([128, 2, 128], BF16, tag="wq0a", name="wq0a")
            wq0b = cpool.tile([128, 14, 128], BF16, tag="wq0b", name="wq0b")
            wq_slab = [None] + [
                cpool.tile([128, NCHUNK, 128], BF16, tag=f"wq{s}", name=f"wq{s}")
                for s in range(1, 8)
            ]
            wv_sb = cpool.tile([128, NCHUNK, 512], BF16, tag="wv")
            wp_sb = cpool.tile([128, HL, DIM], BF16, tag="wp")
            cos_sb = cpool.tile([128, T], BF16, tag="cos")
            sin_sb = cpool.tile([128, T], BF16, tag="sin")
            tri_sb = cpool.tile([128, 128], BF16, tag="tri")
            ones_sb = cpool.tile([128, 128], F16, tag="ones")
            q_sb = cpool.tile([128, HL * T], BF16, tag="q")
            k_sb = cpool.tile([128, HL * T], BF16, tag="k")
            v_sb = cpool.tile([128, NTT * HL * D], BF16, tag="v")
            o_sb = cpool.tile([128, HL * T], BF16, tag="o")

            xbs = {}

            def load_xb(w):
                xb = xpool.tile([128, NCHUNK, 512], BF16, tag="xb", name="xb")
                nc.sync.dma_start(xb[:], xPd[:, w])
                xbs[w] = xb

            # Startup DMAs, first-needed first, split across both HWDGE
            # queues (sync + scalar); xb0 in 2-chunk pieces alternating
            # queues so set-0's chunk loop rarely outruns arrival; half-height
            # rope tables on the gpsimd SWDGE, mirrored on-chip by DVE.
            xb0 = xpool.tile([128, NCHUNK, 512], BF16, tag="xb", name="xb0")
            xbs[0] = xb0
            nc.scalar.dma_start(wq0a[:], wqPd[:, 0, 0:2, :])
            nc.sync.dma_start(xb0[:, 0:1, :], xPd[:, 0, 0:1, :])
            nc.gpsimd.dma_start(cos_sb[0:64, :], cosT[:, :])
            nc.gpsimd.dma_start(sin_sb[64:128, :], sinT[:, :])
            nc.scalar.dma_start(wq0b[:], wqPd[:, 0, 2:16, :])
            nc.sync.dma_start(xb0[:, 1:3, :], xPd[:, 0, 1:3, :])
            nc.sync.dma_start(xb0[:, 3:6, :], xPd[:, 0, 3:6, :])
            nc.scalar.dma_start(xb0[:, 6:8, :], xPd[:, 0, 6:8, :])
            nc.sync.dma_start(xb0[:, 8:10, :], xPd[:, 0, 8:10, :])
            nc.scalar.dma_start(wq_slab[1][:], wqPd[:, 1])
            nc.sync.dma_start(xb0[:, 10:12, :], xPd[:, 0, 10:12, :])
            nc.scalar.dma_start(xb0[:, 12:14, :], xPd[:, 0, 12:14, :])
            nc.sync.dma_start(xb0[:, 14:16, :], xPd[:, 0, 14:16, :])
            nc.gpsimd.dma_start(tri_sb[:], tri[:, :])
            nc.gpsimd.dma_start(ones_sb[:], ones[:, :])
            nc.scalar.dma_start(wq_slab[2][:], wqPd[:, 2])
            nc.sync.dma_start(wq_slab[3][:], wqPd[:, 3])
            nc.sync.dma_start(wv_sb[:, 0:8, :], wvPd[:, 0:8, :])
            nc.scalar.dma_start(wv_sb[:, 8:16, :], wvPd[:, 8:16, :])
            nc.sync.dma_start(wq_slab[4][:], wqPd[:, 4])
            nc.scalar.dma_start(wq_slab[5][:], wqPd[:, 5])
            nc.sync.dma_start(wq_slab[6][:], wqPd[:, 6])
            nc.scalar.dma_start(wq_slab[7][:], wqPd[:, 7])
            # mirror the half-height tables on DVE (cos rows repeat; sin rows
            # 0-63 carry the folded rotate-half sign)
            nc.vector.tensor_copy(cos_sb[64:128, :], cos_sb[0:64, :])
            nc.vector.tensor_scalar_mul(sin_sb[0:64, :], sin_sb[64:128, :], -1.0)

            def qkv_bigs(w):
                """[('q'|'k'|'v', idx, closure)] — 8 qk sets then 4 v sets."""
                wsl = slice(w * 512, (w + 1) * 512)

                def qk_set(grp, j):
                    def run():
                        xb = xbs[w]
                        dst = q_sb if grp == 0 else k_sb
                        ps = psA.tile([128, 512], F32, tag="ps", name="ps")
                        s = grp * 4 + j
                        for c in range(NCHUNK):
                            if s == 0:
                                slab = wq0a[:, c, :] if c < 2 else wq0b[:, c - 2, :]
                            else:
                                slab = wq_slab[s][:, c, :]
                            nc.tensor.matmul(
                                ps[:],
                                slab,
                                xb[:, c, :],
                                start=(c == 0),
                                stop=(c == NCHUNK - 1),
                            )
                        # fused RoPE evacuation: dst = ps*cos + rot(ps)*sin
                        sl = slice(j * T + w * 512, j * T + (w + 1) * 512)
                        rot = rotpool.tile([128, 512], BF16, tag="rot", name="rot")
                        nc.vector.tensor_mul(
                            rot[0:64, :], ps[64:128, :], sin_sb[0:64, wsl]
                        )
                        nc.vector.tensor_mul(
                            rot[64:128, :], ps[0:64, :], sin_sb[64:128, wsl]
                        )
                        nc.vector.tensor_mul(dst[:, sl], ps[:], cos_sb[:, wsl])
                        nc.vector.tensor_add(dst[:, sl], dst[:, sl], rot[:])

                    return run

                def v_set(ttl):
                    def run():
                        xb = xbs[w]
                        ttg = w * 4 + ttl
                        ps = psA.tile([128, 512], F32, tag="ps", name="ps")
                        for c in range(NCHUNK):
                            nc.tensor.matmul(
                                ps[:],
                                xb[:, c, ttl * 128 : (ttl + 1) * 128],
                                wv_sb[:, c, :],
                                start=(c == 0),
                                stop=(c == NCHUNK - 1),
                            )
                        nc.scalar.copy(v_sb[:, ttg * 512 : (ttg + 1) * 512], ps[:])

                    return run

                return (
                    [("q", j, qk_set(0, j)) for j in range(HL)]
                    + [("v", ttl, v_set(ttl)) for ttl in range(4)]
                    + [("k", j, qk_set(1, j)) for j in range(HL)]
                )

            def attn_fillers(w, pos):
                """Per-(head, key-tile) closures + their readiness (number of
                this window's qkv bigs that must have been emitted first).
                All of a head-window's pt tiles chain-accumulate into one
                fp16 acc on DVE (one add per unit); a single 512-col
                all-ones matmul per head produces the softmax denominators,
                emitted one unit into the next head so the single-bank
                rowsum tile's WAR on the previous reciprocal is covered."""
                nkt = 4 * w + 4
                OD = 4 * w            # off-diagonal tiles per head
                nu = HL * nkt
                state = {
                    "pend": [],
                    "issued": 0,
                    "oT": {},
                    "pt0": None,
                    "acc": {},
                }

                def geom(kt):
                    if kt < OD:
                        return 512 * w, 512, False
                    q0 = 128 * kt
                    return q0, 512 * (w + 1) - 128 * kt, True

                def issue_score(u):
                    h, kt = divmod(u, nkt)
                    hq = h * T
                    q0, n, diag = geom(kt)
                    st = psS.tile([128, 512], F32, tag="st", name="st")
                    nc.tensor.matmul(
                        st[:, :n],
                        k_sb[:, hq + kt * 128 : hq + (kt + 1) * 128],
                        q_sb[:, hq + q0 : hq + q0 + n],
                        start=True,
                        stop=True,
                    )
                    pt = ptpool.tile([128, 512], BF16, tag="pt", name="pt")
                    nc.scalar.activation(
                        pt[:, :n], st[:, :n], Exp, bias=0.0, scale=SCALE
                    )
                    if diag:
                        nc.vector.tensor_mul(pt[:, 0:128], pt[:, 0:128], tri_sb[:])
                    return pt, q0, n

                DEPTH = 3

                def flush_rs(h):
                    # single rowsum matmul for head h, then the off-PE
                    # normalization chain
                    rs = psR.tile([128, 512], F32, tag="rs", name="rs")
                    nc.tensor.matmul(
                        rs[:],
                        ones_sb[:],
                        state["acc"].pop(h)[:],
                        start=True,
                        stop=True,
                    )
                    hq = h * T
                    rcp = rcppool.tile([128, 512], F32, tag="rcp", name="rcp")
                    nc.vector.reciprocal_approx_fast(rcp[:], rs[:])
                    nc.vector.tensor_mul(
                        o_sb[:, hq + w * 512 : hq + (w + 1) * 512],
                        state["oT"].pop(h)[:],
                        rcp[:],
                    )

                def unit(u):
                    def run():
                        h, kt = divmod(u, nkt)
                        while state["issued"] < min(u + DEPTH + 1, nu):
                            state["pend"].append(issue_score(state["issued"]))
                            state["issued"] += 1
                        if kt == 0:
                            state["oT"][h] = psO.tile(
                                [128, 512], F32, tag="oT", name="oT"
                            )
                        pt, q0, n = state["pend"].pop(0)
                        off = q0 - 512 * w
                        nc.tensor.matmul(
                            state["oT"][h][:, off:512],
                            v_sb[:, kt * 512 + h * 128 : kt * 512 + (h + 1) * 128],
                            pt[:, :n],
                            start=(kt == 0),
                            stop=(kt == nkt - 1),
                        )
                        # denominator accumulation (kt = 0 always has n = 512)
                        if kt == 0:
                            state["pt0"] = pt
                        elif kt == 1:
                            acc = rsapool.tile(
                                [128, 512], F16, tag="acc", name="acc"
                            )
                            if n == 512:
                                nc.vector.tensor_add(
                                    acc[:], state["pt0"][:], pt[:]
                                )
                            else:
                                nc.vector.tensor_copy(
                                    acc[:, 0:off], state["pt0"][:, 0:off]
                                )
                                nc.vector.tensor_add(
                                    acc[:, off:512],
                                    state["pt0"][:, off:512],
                                    pt[:, :n],
                                )
                            state["acc"][h] = acc
                        else:
                            acc = state["acc"][h]
                            nc.vector.tensor_add(
                                acc[:, off:512], acc[:, off:512], pt[:, :n]
                            )
                        if kt == 1 and h > 0:
                            flush_rs(h - 1)
                        if u == nu - 1:
                            flush_rs(h)

                    return run

                def score_ra(u):
                    h, kt = divmod(u, nkt)
                    if kt < OD:
                        return pos[("q", h)] + 1
                    return pos[("k", h)] + 1

                def pv_ra(u):
                    h, kt = divmod(u, nkt)
                    if kt < OD:
                        return 0
                    return pos[("v", kt - OD)] + 1

                fillers = []
                ready = []
                for u in range(nu):
                    ra = pv_ra(u)
                    for uu in range(u, min(u + DEPTH + 1, nu)):
                        ra = max(ra, score_ra(uu))
                    fillers.append(unit(u))
                    ready.append(ra)
                return fillers, ready

            def proj_bigs(w, pools=None, alt_dma=False):
                out = []

                def unit(tt, nwi, pool, ptag, dve_evac, dmae):
                    def run():
                        yps = pool.tile([128, 512], F32, tag=ptag, name="yps")
                        for hh in range(HL):
                            nc.tensor.matmul(
                                yps[:],
                                o_sb[:, hh * T + tt * 128 : hh * T + (tt + 1) * 128],
                                wp_sb[:, hh, nwi * 512 : (nwi + 1) * 512],
                                start=(hh == 0),
                                stop=(hh == HL - 1),
                            )
                        ysb = ypool.tile([128, 512], F32, tag="ysb", name="ysb")
                        if dve_evac:
                            nc.vector.tensor_copy(ysb[:], yps[:])
                        else:
                            nc.scalar.copy(ysb[:], yps[:])
                        dmae.dma_start(
                            y[tt * 128 : (tt + 1) * 128, nwi * 512 : (nwi + 1) * 512],
                            ysb[:],
                        )

                    return run

                i = 0
                for tt in range(4 * w, 4 * w + 4):
                    for nwi in range(DIM // 512):
                        if pools:
                            pool, ptag = pools[i % len(pools)]
                        else:
                            pool, ptag = psA, "ps"
                        # in the final batch (attention over) split the
                        # evacuations across ACT and DVE and put each unit's
                        # DMA config on the opposite sequencer so neither
                        # sequencer serializes copy + trigger
                        dve = alt_dma and (i % 2 == 1)
                        dmae = nc.scalar if dve else nc.sync
                        out.append(
                            ("p", i, unit(tt, nwi, pool, ptag, dve, dmae))
                        )
                        i += 1
                return out

            def weave(bigs, fillers, ready):
                nb = len(bigs)
                nf = len(fillers)
                done = 0
                for i, (_, _, b) in enumerate(bigs):
                    b()
                    # cap the per-big burst so late-ready units don't flood
                    # the exp engine all at once
                    target = min(int(round(nf * (i + 1) / nb)), done + 4)
                    while done < nf and done < target and ready[done] <= i + 1:
                        fillers[done]()
                        done += 1
                while done < nf:
                    fillers[done]()
                    done += 1

            for w in range(NW):
                bigs = qkv_bigs(w)
                if w + 1 < NW:
                    # prefetch next window's x once the startup crunch is over
                    bigs.insert(6, ("x", w + 1, lambda wn=w + 1: load_xb(wn)))
                if w == 0:
                    bigs.append(
                        ("wp", 0, lambda: nc.sync.dma_start(wp_sb[:], wpPd[:]))
                    )
                if w >= 1:
                    bigs += proj_bigs(w - 1)
                pos = {(lbl, idx): i for i, (lbl, idx, _) in enumerate(bigs)}
                fillers, ready = attn_fillers(w, pos)
                weave(bigs, fillers, ready)
            # after the last window's attention, all PSUM pools are free:
            # cycle the final proj units across them so the evacuation chain
            # never blocks the PE on a bank WAR
            last = proj_bigs(NW - 1)
            for _, _, fn in last[:-1]:
                fn()
            # final unit split into column halves on both HWDGE queues so the
            # exposed tail evacuation + y DMA is halved
            tt, nwi = NTT - 1, DIM // 512 - 1
            yps = psA.tile([128, 512], F32, tag="ps", name="yps")
            for hh in range(HL):
                nc.tensor.matmul(
                    yps[:],
                    o_sb[:, hh * T + tt * 128 : hh * T + (tt + 1) * 128],
                    wp_sb[:, hh, nwi * 512 : (nwi + 1) * 512],
                    start=(hh == 0),
                    stop=(hh == HL - 1),
                )
            ysb = ypool.tile([128, 512], F32, tag="ysb", name="ysb")
            r0 = slice(tt * 128, (tt + 1) * 128)
            nc.scalar.copy(ysb[:, 0:256], yps[:, 0:256])
            nc.sync.dma_start(
                y[r0, nwi * 512 : nwi * 512 + 256], ysb[:, 0:256]
            )
            nc.scalar.copy(ysb[:, 256:512], yps[:, 256:512])
            nc.scalar.dma_start(
                y[r0, nwi * 512 + 256 : (nwi + 1) * 512], ysb[:, 256:512]
            )

    nc.compile()
    return nc


def _rope_tables():
    inv_freq = (
        1.0 / (10000.0 ** (np.arange(0, D, 2, dtype=np.float32) / np.float32(D)))
    ).astype(np.float32)
    tpos = np.arange(T, dtype=np.float32)
    freqs = tpos[:, None] * inv_freq[None, :]  # (T, 64)
    cosT = np.ascontiguousarray(np.cos(freqs).T)  # (64, T)
    sinT = np.ascontiguousarray(np.sin(freqs).T)
    return (
        cosT.astype(ml_dtypes.bfloat16),
        sinT.astype(ml_dtypes.bfloat16),
    )


def make_in_maps(x, W_qkv, W_proj):
    cosT, sinT = _rope_tables()
    tri = (np.arange(128)[None, :] >= np.arange(128)[:, None]).astype(
        ml_dtypes.bfloat16
    )
    tri = np.ascontiguousarray(tri)
    ones = np.ones((128, 128), dtype=np.float16)

    xPs = {}
    for b in range(B):
        xt = np.ascontiguousarray(x[b].T).astype(ml_dtypes.bfloat16)  # (dim, T)
        xPs[b] = np.ascontiguousarray(
            xt.reshape(NCHUNK, 128, NW, 512).transpose(1, 2, 0, 3)
        )

    wqPs, wvPs, wpPs = {}, {}, {}
    for g in range(4):
        Wq = W_qkv[512 * g : 512 * (g + 1)]
        Wk = W_qkv[2048 + 512 * g : 2048 + 512 * (g + 1)]
        Wv = W_qkv[4096 + 512 * g : 4096 + 512 * (g + 1)]
        Wc = np.concatenate([Wq, Wk, Wv], axis=0)  # (1536, 2048)
        A = (
            np.ascontiguousarray(Wc.T)
            .astype(ml_dtypes.bfloat16)
            .reshape(NCHUNK, 128, 1536)
            .transpose(1, 0, 2)
        )  # [p, c, e]
        wqPs[g] = np.ascontiguousarray(
            A[:, :, :1024].reshape(128, NCHUNK, 8, 128).transpose(0, 2, 1, 3)
        )  # [p, s, c, e]
        wvPs[g] = np.ascontiguousarray(A[:, :, 1024:])  # [p, c, e512]
        wpPs[g] = np.ascontiguousarray(
            np.ascontiguousarray(W_proj[:, 512 * g : 512 * (g + 1)].T)
            .astype(ml_dtypes.bfloat16)
            .reshape(HL, 128, DIM)
            .transpose(1, 0, 2)
        )  # [p, h, n]

    in_maps = []
    for c in range(NCORES):
        b, g = divmod(c, 4)
        in_maps.append(
            {
                "xP": xPs[b],
                "wqP": wqPs[g],
                "wvP": wvPs[g],
                "wpP": wpPs[g],
                "cosT": cosT,
                "sinT": sinT,
                "tri": tri,
                "ones": ones,
            }
        )
    return in_maps


def kernel(x, W_qkv, W_proj):
    global LAST_RESULTS
    x = np.asarray(x, dtype=np.float32)
    W_qkv = np.asarray(W_qkv, dtype=np.float32)
    W_proj = np.asarray(W_proj, dtype=np.float32)
    assert x.shape == (B, T, DIM) and W_qkv.shape == (3 * H * D, DIM)

    if "nc" not in _CACHE:
        _CACHE["nc"] = _build_module()
    nc = _CACHE["nc"]

    in_maps = make_in_maps(x, W_qkv, W_proj)
    trace = os.environ.get("KERNEL_TRACE", "0") == "1"
    res = bass_utils.run_bass_kernel_spmd(
        nc, in_maps, core_ids=list(range(NCORES)), trace=trace
    )
    LAST_RESULTS = res
    y = np.zeros((B, T, DIM), dtype=np.float32)
    for c in range(NCORES):
        y[c // 4] += res.results[c]["y"]
    return y


# revision 30
# speedup vs baseline: 1.0166x; 1.0073x over previous
"""Causal self-attention (B=2, T=2048, dim=2048, H=16, D=128) on 8 trn2 NeuronCores.

Sharding: data-parallel over batch (2 groups of 4 cores), tensor-parallel over
heads within a group (4 heads/core).  Each core computes its heads' QKV
projection (x @ Wqkv_part^T), RoPE, causal attention, and a partial output
projection against its W_proj column block; the host sums the 4 partials per
batch element.

v8 schedule (all matmul operands bf16, fp32 accumulation): one readiness-aware
software pipeline — attention units of window w are woven between the QKV
matmul sets of the SAME window and the proj units of window w-1, so the
ScalarE exp stream always drains under QKV/proj PE work:
  - all DRAM operands host-packed so every DMA lands in >=4KB contiguous
    per-partition lines; startup loads split across the two HWDGE queues
    (sync + scalar) in first-use order; first weight slab split so the
    first matmul gates on 128KB; rope tables shipped as 64 rows and
    mirrored on-chip (rows 0-63 == rows 64-127 up to the rotate sign).
  - RoPE fused into the QKV PSUM evacuation on DVE: rot halves are
    partition-offset multiplies against the sign-folded sin table, then
    dst = ps*cos + rot.  No PE swap matmul, no separate evacuation cast.
  - scores pipelined depth-3: PE score matmul -> ScalarE exp (bf16) -> PE
    PV.  Each head-window's pt tiles chain-accumulate into one fp16 tile
    on DVE (one add per unit), so the PE rowsum is a single 512-col
    all-ones matmul per head (denominators replicated across partitions).
  - softmax normalization off the PE: reciprocal_approx_fast + multiply (DVE).
  - QKV/proj PSUM evacuation on DVE/ScalarE; y DMA'd per window.
"""

import os

import numpy as np
import ml_dtypes

import concourse.bass as bass
import concourse.bacc as bacc
import concourse.tile as tile
import concourse.mybir as mybir
from concourse import bass_utils

BF16 = mybir.dt.bfloat16
F16 = mybir.dt.float16
F32 = mybir.dt.float32

B, T, DIM = 2, 2048, 2048
H, D = 16, 128
HL = 4                   # heads per core
NCORES = 8
NCHUNK = DIM // 128      # 16 contraction chunks
NW = T // 512            # 4 query windows
NTT = T // 128           # 16 token tiles
SCALE = 1.0 / float(np.sqrt(D))

_CACHE = {}
LAST_RESULTS = None


def _build_module():
    nc = bacc.Bacc("TRN2", target_bir_lowering=False, debug=False)
    # host-packed layouts: partition dim first, contiguous per-partition lines
    xPd = nc.dram_tensor("xP", (128, NW, NCHUNK, 512), BF16, kind="ExternalInput")
    wqPd = nc.dram_tensor("wqP", (128, 8, NCHUNK, 128), BF16, kind="ExternalInput")
    wvPd = nc.dram_tensor("wvP", (128, NCHUNK, 512), BF16, kind="ExternalInput")
    wpPd = nc.dram_tensor("wpP", (128, HL, DIM), BF16, kind="ExternalInput")
    cosT = nc.dram_tensor("cosT", (64, T), BF16, kind="ExternalInput")
    sinT = nc.dram_tensor("sinT", (64, T), BF16, kind="ExternalInput")
    tri = nc.dram_tensor("tri", (128, 128), BF16, kind="ExternalInput")
    ones = nc.dram_tensor("ones", (128, 128), F16, kind="ExternalInput")
    y = nc.dram_tensor("y", (T, DIM), F32, kind="ExternalOutput")

    Exp = mybir.ActivationFunctionType.Exp

    with tile.TileContext(nc) as tc:
        with (
            tc.tile_pool(name="const", bufs=1) as cpool,
            tc.tile_pool(name="xp", bufs=2) as xpool,
            tc.tile_pool(name="rotp", bufs=3) as rotpool,
            tc.tile_pool(name="ptp", bufs=6) as ptpool,
            tc.tile_pool(name="rsap", bufs=2) as rsapool,
            tc.tile_pool(name="rcpp", bufs=2) as rcppool,
            tc.tile_pool(name="yp", bufs=3) as ypool,
            tc.tile_pool(name="yp2", bufs=3) as ypool2,
            tc.tile_pool(name="psA", bufs=2, space="PSUM") as psA,
            tc.tile_pool(name="psS", bufs=3, space="PSUM") as psS,
            tc.tile_pool(name="psO", bufs=2, space="PSUM") as psO,
            tc.tile_pool(name="psR", bufs=1, space="PSUM") as psR,
        ):
            # per-set weight slabs: dependency granularity is per-tile, so a
            # qk set only waits for its own slab's DMA; slab 0 split so the
            # first matmul gates on a 128KB transfer
            wq0a = cpool.tile([128, 2, 128], BF16, tag="wq0a", name="wq0a")
            wq0b = cpool.tile([128, 14, 128], BF16, tag="wq0b", name="wq0b")
            wq_slab = [None] + [
                cpool.tile([128, NCHUNK, 128], BF16, tag=f"wq{s}", name=f"wq{s}")
                for s in range(1, 8)
            ]
            wv_sb = cpool.tile([128, NCHUNK, 512], BF16, tag="wv")
            wp_sb = cpool.tile([128, HL, DIM], BF16, tag="wp")
            cos_sb = cpool.tile([128, T], BF16, tag="cos")
            sin_sb = cpool.tile([128, T], BF16, tag="sin")
            tri_sb = cpool.tile([128, 128], BF16, tag="tri")
            ones_sb = cpool.tile([128, 128], F16, tag="ones")
            q_sb = cpool.tile([128, HL * T], BF16, tag="q")
            k_sb = cpool.tile([128, HL * T], BF16, tag="k")
            v_sb = cpool.tile([128, NTT * HL * D], BF16, tag="v")
            o_sb = cpool.tile([128, HL * T], BF16, tag="o")

            xbs = {}

            def load_xb(w):
                xb = xpool.tile([128, NCHUNK, 512], BF16, tag="xb", name="xb")
                nc.sync.dma_start(xb[:], xPd[:, w])
                xbs[w] = xb

            # Startup DMAs, first-needed first, split across both HWDGE
            # queues (sync + scalar); xb0 in 2-chunk pieces alternating
            # queues so set-0's chunk loop rarely outruns arrival; half-height
            # rope tables on the gpsimd SWDGE, mirrored on-chip by DVE.
            xb0 = xpool.tile([128, NCHUNK, 512], BF16, tag="xb", name="xb0")
            xbs[0] = xb0
            nc.scalar.dma_start(wq0a[:], wqPd[:, 0, 0:2, :])
            nc.sync.dma_start(xb0[:, 0:1, :], xPd[:, 0, 0:1, :])
            nc.gpsimd.dma_start(cos_sb[0:64, :], cosT[:, :])
            nc.gpsimd.dma_start(sin_sb[64:128, :], sinT[:, :])
            nc.scalar.dma_start(wq0b[:], wqPd[:, 0, 2:16, :])
            nc.sync.dma_start(xb0[:, 1:3, :], xPd[:, 0, 1:3, :])
            nc.sync.dma_start(xb0[:, 3:6, :], xPd[:, 0, 3:6, :])
            nc.scalar.dma_start(xb0[:, 6:8, :], xPd[:, 0, 6:8, :])
            nc.sync.dma_start(xb0[:, 8:10, :], xPd[:, 0, 8:10, :])
            nc.scalar.dma_start(wq_slab[1][:], wqPd[:, 1])
            nc.sync.dma_start(xb0[:, 10:12, :], xPd[:, 0, 10:12, :])
            nc.scalar.dma_start(xb0[:, 12:14, :], xPd[:, 0, 12:14, :])
            nc.sync.dma_start(xb0[:, 14:16, :], xPd[:, 0, 14:16, :])
            nc.gpsimd.dma_start(tri_sb[:], tri[:, :])
            nc.gpsimd.dma_start(ones_sb[:], ones[:, :])
            nc.scalar.dma_start(wq_slab[2][:], wqPd[:, 2])
            nc.sync.dma_start(wq_slab[3][:], wqPd[:, 3])
            nc.sync.dma_start(wv_sb[:, 0:8, :], wvPd[:, 0:8, :])
            nc.scalar.dma_start(wv_sb[:, 8:16, :], wvPd[:, 8:16, :])
            nc.sync.dma_start(wq_slab[4][:], wqPd[:, 4])
            nc.scalar.dma_start(wq_slab[5][:], wqPd[:, 5])
            nc.sync.dma_start(wq_slab[6][:], wqPd[:, 6])
            nc.scalar.dma_start(wq_slab[7][:], wqPd[:, 7])
            # mirror the half-height tables on DVE (cos rows repeat; sin rows
            # 0-63 carry the folded rotate-half sign)
            nc.vector.tensor_copy(cos_sb[64:128, :], cos_sb[0:64, :])
            nc.vector.tensor_scalar_mul(sin_sb[0:64, :], sin_sb[64:128, :], -1.0)

            def qkv_bigs(w):
                """[('q'|'k'|'v', idx, closure)] — 8 qk sets then 4 v sets."""
                wsl = slice(w * 512, (w + 1) * 512)

                def qk_set(grp, j):
                    def run():
                        xb = xbs[w]
                        dst = q_sb if grp == 0 else k_sb
                        ps = psA.tile([128, 512], F32, tag="ps", name="ps")
                        s = grp * 4 + j
                        for c in range(NCHUNK):
                            if s == 0:
                                slab = wq0a[:, c, :] if c < 2 else wq0b[:, c - 2, :]
                            else:
                                slab = wq_slab[s][:, c, :]
                            nc.tensor.matmul(
                                ps[:],
                                slab,
                                xb[:, c, :],
                                start=(c == 0),
                                stop=(c == NCHUNK - 1),
                            )
                        # fused RoPE evacuation: dst = ps*cos + rot(ps)*sin
                        sl = slice(j * T + w * 512, j * T + (w + 1) * 512)
                        rot = rotpool.tile([128, 512], BF16, tag="rot", name="rot")
                        nc.vector.tensor_mul(
                            rot[0:64, :], ps[64:128, :], sin_sb[0:64, wsl]
                        )
                        nc.vector.tensor_mul(
                            rot[64:128, :], ps[0:64, :], sin_sb[64:128, wsl]
                        )
                        nc.vector.tensor_mul(dst[:, sl], ps[:], cos_sb[:, wsl])
                        nc.vector.tensor_add(dst[:, sl], dst[:, sl], rot[:])

                    return run

                def v_set(ttl):
                    def run():
                        xb = xbs[w]
                        ttg = w * 4 + ttl
                        ps = psA.tile([128, 512], F32, tag="ps", name="ps")
                        for c in range(NCHUNK):
                            nc.tensor.matmul(
                                ps[:],
                                xb[:, c, ttl * 128 : (ttl + 1) * 128],
                                wv_sb[:, c, :],
                                start=(c == 0),
                                stop=(c == NCHUNK - 1),
                            )
                        nc.scalar.copy(v_sb[:, ttg * 512 : (ttg + 1) * 512], ps[:])

                    return run

                return (
                    [("q", j, qk_set(0, j)) for j in range(HL)]
                    + [("v", ttl, v_set(ttl)) for ttl in range(4)]
                    + [("k", j, qk_set(1, j)) for j in range(HL)]
                )

            def attn_fillers(w, pos):
                """Per-(head, key-tile) closures + their readiness (number of
                this window's qkv bigs that must have been emitted first).
                All of a head-window's pt tiles chain-accumulate into one
                fp16 acc on DVE (one add per unit); a single 512-col
                all-ones matmul per head produces the softmax denominators,
                emitted one unit into the next head so the single-bank
                rowsum tile's WAR on the previous reciprocal is covered."""
                nkt = 4 * w + 4
                OD = 4 * w            # off-diagonal tiles per head
                nu = HL * nkt
                state = {
                    "pend": [],
                    "issued": 0,
                    "oT": {},
                    "pt0": None,
                    "acc": {},
                }

                def geom(kt):
                    if kt < OD:
                        return 512 * w, 512, False
                    q0 = 128 * kt
                    return q0, 512 * (w + 1) - 128 * kt, True

                def issue_score(u):
                    h, kt = divmod(u, nkt)
                    hq = h * T
                    q0, n, diag = geom(kt)
                    st = psS.tile([128, 512], F32, tag="st", name="st")
                    nc.tensor.matmul(
                        st[:, :n],
                        k_sb[:, hq + kt * 128 : hq + (kt + 1) * 128],
                        q_sb[:, hq + q0 : hq + q0 + n],
                        start=True,
                        stop=True,
                    )
                    pt = ptpool.tile([128, 512], BF16, tag="pt", name="pt")
                    nc.scalar.activation(
                        pt[:, :n], st[:, :n], Exp, bias=0.0, scale=SCALE
                    )
                    if diag:
                        nc.vector.tensor_mul(pt[:, 0:128], pt[:, 0:128], tri_sb[:])
                    return pt, q0, n

                DEPTH = 3

                def flush_rs(h):
                    # single rowsum matmul for head h, then the off-PE
                    # normalization chain
                    rs = psR.tile([128, 512], F32, tag="rs", name="rs")
                    nc.tensor.matmul(
                        rs[:],
                        ones_sb[:],
                        state["acc"].pop(h)[:],
                        start=True,
                        stop=True,
                    )
                    hq = h * T
                    rcp = rcppool.tile([128, 512], F32, tag="rcp", name="rcp")
                    nc.vector.reciprocal_approx_fast(rcp[:], rs[:])
                    nc.vector.tensor_mul(
                        o_sb[:, hq + w * 512 : hq + (w + 1) * 512],
                        state["oT"].pop(h)[:],
                        rcp[:],
                    )

                def unit(u):
                    def run():
                        h, kt = divmod(u, nkt)
                        while state["issued"] < min(u + DEPTH + 1, nu):
                            state["pend"].append(issue_score(state["issued"]))
                            state["issued"] += 1
                        if kt == 0:
                            state["oT"][h] = psO.tile(
                                [128, 512], F32, tag="oT", name="oT"
                            )
                        pt, q0, n = state["pend"].pop(0)
                        off = q0 - 512 * w
                        nc.tensor.matmul(
                            state["oT"][h][:, off:512],
                            v_sb[:, kt * 512 + h * 128 : kt * 512 + (h + 1) * 128],
                            pt[:, :n],
                            start=(kt == 0),
                            stop=(kt == nkt - 1),
                        )
                        # denominator accumulation (kt = 0 always has n = 512)
                        if kt == 0:
                            state["pt0"] = pt
                        elif kt == 1:
                            acc = rsapool.tile(
                                [128, 512], F16, tag="acc", name="acc"
                            )
                            if n == 512:
                                nc.vector.tensor_add(
                                    acc[:], state["pt0"][:], pt[:]
                                )
                            else:
                                nc.vector.tensor_copy(
                                    acc[:, 0:off], state["pt0"][:, 0:off]
                                )
                                nc.vector.tensor_add(
                                    acc[:, off:512],
                                    state["pt0"][:, off:512],
                                    pt[:, :n],
                                )
                            state["acc"][h] = acc
                        else:
                            acc = state["acc"][h]
                            nc.vector.tensor_add(
                                acc[:, off:512], acc[:, off:512], pt[:, :n]
                            )
                        if kt == 1 and h > 0:
                            flush_rs(h - 1)
                        if u == nu - 1:
                            flush_rs(h)

                    return run

                def score_ra(u):
                    h, kt = divmod(u, nkt)
                    if kt < OD:
                        return pos[("q", h)] + 1
                    return pos[("k", h)] + 1

                def pv_ra(u):
                    h, kt = divmod(u, nkt)
                    if kt < OD:
                        return 0
                    return pos[("v", kt - OD)] + 1

                fillers = []
                ready = []
                for u in range(nu):
                    ra = pv_ra(u)
                    for uu in range(u, min(u + DEPTH + 1, nu)):
                        ra = max(ra, score_ra(uu))
                    fillers.append(unit(u))
                    ready.append(ra)
                return fillers, ready

            def proj_bigs(w, pools=None, alt_dma=False):
                out = []

                def unit(tt, nwi, pool, ptag, dve_evac, dmae):
                    def run():
                        yps = pool.tile([128, 512], F32, tag=ptag, name="yps")
                        for hh in range(HL):
                            nc.tensor.matmul(
                                yps[:],
                                o_sb[:, hh * T + tt * 128 : hh * T + (tt + 1) * 128],
                                wp_sb[:, hh, nwi * 512 : (nwi + 1) * 512],
                                start=(hh == 0),
                                stop=(hh == HL - 1),
                            )
                        ysb = ypool.tile([128, 512], F32, tag="ysb", name="ysb")
                        if dve_evac:
                            nc.vector.tensor_copy(ysb[:], yps[:])
                        else:
                            nc.scalar.copy(ysb[:], yps[:])
                        dmae.dma_start(
                            y[tt * 128 : (tt + 1) * 128, nwi * 512 : (nwi + 1) * 512],
                            ysb[:],
                        )

                    return run

                i = 0
                for tt in range(4 * w, 4 * w + 4):
                    for nwi in range(DIM // 512):
                        if pools:
                            pool, ptag = pools[i % len(pools)]
                        else:
                            pool, ptag = psA, "ps"
                        # in the final batch (attention over) split the
                        # evacuations across ACT and DVE and put each unit's
                        # DMA config on the opposite sequencer so neither
                        # sequencer serializes copy + trigger
                        dve = alt_dma and (i % 2 == 1)
                        dmae = nc.scalar if dve else nc.sync
                        out.append(
                            ("p", i, unit(tt, nwi, pool, ptag, dve, dmae))
                        )
                        i += 1
                return out

            def weave(bigs, fillers, ready):
                nb = len(bigs)
                nf = len(fillers)
                done = 0
                for i, (_, _, b) in enumerate(bigs):
                    b()
                    # cap the per-big burst so late-ready units don't flood
                    # the exp engine all at once
                    target = min(int(round(nf * (i + 1) / nb)), done + 4)
                    while done < nf and done < target and ready[done] <= i + 1:
                        fillers[done]()
                        done += 1
                while done < nf:
                    fillers[done]()
                    done += 1

            for w in range(NW):
                bigs = qkv_bigs(w)
                if w + 1 < NW:
                    # prefetch next window's x once the startup crunch is over
                    bigs.insert(6, ("x", w + 1, lambda wn=w + 1: load_xb(wn)))
                if w == 0:
                    bigs.append(
                        ("wp", 0, lambda: nc.sync.dma_start(wp_sb[:], wpPd[:]))
                    )
                if w >= 1:
                    bigs += proj_bigs(w - 1)
                pos = {(lbl, idx): i for i, (lbl, idx, _) in enumerate(bigs)}
                fillers, ready = attn_fillers(w, pos)
                weave(bigs, fillers, ready)
            # after the last window's attention, all PSUM pools are free:
            # cycle the final proj units across them so the evacuation chain
            # never blocks the PE on a bank WAR
            # final batch: pair adjacent nwi units into one [128,1024] ysb
            # tile - one DMA per pair, copies split across ACT and DVE,
            # pair DMAs alternating both HWDGE queues (exp stream is over)
            def final_pair(tt, nwi0, dmae):
                ysb2 = ypool2.tile([128, 1024], F32, tag="ysb2", name="ysb2")
                for half in range(2):
                    nwi = nwi0 + half
                    yps = psA.tile([128, 512], F32, tag="ps", name="yps")
                    for hh in range(HL):
                        nc.tensor.matmul(
                            yps[:],
                            o_sb[:, hh * T + tt * 128 : hh * T + (tt + 1) * 128],
                            wp_sb[:, hh, nwi * 512 : (nwi + 1) * 512],
                            start=(hh == 0),
                            stop=(hh == HL - 1),
                        )
                    if half == 0:
                        nc.scalar.copy(ysb2[:, 0:512], yps[:])
                    else:
                        nc.vector.tensor_copy(ysb2[:, 512:1024], yps[:])
                dmae.dma_start(
                    y[tt * 128 : (tt + 1) * 128, nwi0 * 512 : (nwi0 + 2) * 512],
                    ysb2[:],
                )

            pi = 0
            for tt in range(4 * (NW - 1), 4 * NW):
                for nwi0 in (0, 2):
                    if tt == 4 * NW - 1 and nwi0 == 2:
                        break
                    final_pair(tt, nwi0, nc.scalar if pi % 2 else nc.sync)
                    pi += 1
            # final unit split into column halves on both HWDGE queues so the
            # exposed tail evacuation + y DMA is halved
            tt = NTT - 1
            r0 = slice(tt * 128, (tt + 1) * 128)
            for nwi, half in ((2, 0), (3, 1)):
                yps = psA.tile([128, 512], F32, tag="ps", name="yps")
                for hh in range(HL):
                    nc.tensor.matmul(
                        yps[:],
                        o_sb[:, hh * T + tt * 128 : hh * T + (tt + 1) * 128],
                        wp_sb[:, hh, nwi * 512 : (nwi + 1) * 512],
                        start=(hh == 0),
                        stop=(hh == HL - 1),
                    )
                ysb = ypool.tile([128, 512], F32, tag="ysb", name="ysb")
                if half == 0:
                    nc.vector.tensor_copy(ysb[:], yps[:])
                    nc.sync.dma_start(
                        y[r0, nwi * 512 : (nwi + 1) * 512], ysb[:]
                    )
                else:
                    nc.scalar.copy(ysb[:, 0:256], yps[:, 0:256])
                    nc.scalar.dma_start(
                        y[r0, nwi * 512 : nwi * 512 + 256], ysb[:, 0:256]
                    )
                    nc.vector.tensor_copy(ysb[:, 256:512], yps[:, 256:512])
                    nc.sync.dma_start(
                        y[r0, nwi * 512 + 256 : (nwi + 1) * 512], ysb[:, 256:512]
                    )

    nc.compile()
    return nc


def _rope_tables():
    inv_freq = (
        1.0 / (10000.0 ** (np.arange(0, D, 2, dtype=np.float32) / np.float32(D)))
    ).astype(np.float32)
    tpos = np.arange(T, dtype=np.float32)
    freqs = tpos[:, None] * inv_freq[None, :]  # (T, 64)
    cosT = np.ascontiguousarray(np.cos(freqs).T)  # (64, T)
    sinT = np.ascontiguousarray(np.sin(freqs).T)
    return (
        cosT.astype(ml_dtypes.bfloat16),
        sinT.astype(ml_dtypes.bfloat16),
    )


def make_in_maps(x, W_qkv, W_proj):
    cosT, sinT = _rope_tables()
    tri = (np.arange(128)[None, :] >= np.arange(128)[:, None]).astype(
        ml_dtypes.bfloat16
    )
    tri = np.ascontiguousarray(tri)
    ones = np.ones((128, 128), dtype=np.float16)

    xPs = {}
    for b in range(B):
        xt = np.ascontiguousarray(x[b].T).astype(ml_dtypes.bfloat16)  # (dim, T)
        xPs[b] = np.ascontiguousarray(
            xt.reshape(NCHUNK, 128, NW, 512).transpose(1, 2, 0, 3)
        )

    wqPs, wvPs, wpPs = {}, {}, {}
    for g in range(4):
        Wq = W_qkv[512 * g : 512 * (g + 1)]
        Wk = W_qkv[2048 + 512 * g : 2048 + 512 * (g + 1)]
        Wv = W_qkv[4096 + 512 * g : 4096 + 512 * (g + 1)]
        Wc = np.concatenate([Wq, Wk, Wv], axis=0)  # (1536, 2048)
        A = (
            np.ascontiguousarray(Wc.T)
            .astype(ml_dtypes.bfloat16)
            .reshape(NCHUNK, 128, 1536)
            .transpose(1, 0, 2)
        )  # [p, c, e]
        wqPs[g] = np.ascontiguousarray(
            A[:, :, :1024].reshape(128, NCHUNK, 8, 128).transpose(0, 2, 1, 3)
        )  # [p, s, c, e]
        wvPs[g] = np.ascontiguousarray(A[:, :, 1024:])  # [p, c, e512]
        wpPs[g] = np.ascontiguousarray(
            np.ascontiguousarray(W_proj[:, 512 * g : 512 * (g + 1)].T)
            .astype(ml_dtypes.bfloat16)
            .reshape(HL, 128, DIM)
            .transpose(1, 0, 2)
        )  # [p, h, n]

    in_maps = []
    for c in range(NCORES):
        b, g = divmod(c, 4)
        in_maps.append(
            {
                "xP": xPs[b],
                "wqP": wqPs[g],
                "wvP": wvPs[g],
                "wpP": wpPs[g],
                "cosT": cosT,
                "sinT": sinT,
                "tri": tri,
                "ones": ones,
            }
        )
    return in_maps


def kernel(x, W_qkv, W_proj):
    global LAST_RESULTS
    x = np.asarray(x, dtype=np.float32)
    W_qkv = np.asarray(W_qkv, dtype=np.float32)
    W_proj = np.asarray(W_proj, dtype=np.float32)
    assert x.shape == (B, T, DIM) and W_qkv.shape == (3 * H * D, DIM)

    if "nc" not in _CACHE:
        _CACHE["nc"] = _build_module()
    nc = _CACHE["nc"]

    in_maps = make_in_maps(x, W_qkv, W_proj)
    trace = os.environ.get("KERNEL_TRACE", "0") == "1"
    res = bass_utils.run_bass_kernel_spmd(
        nc, in_maps, core_ids=list(range(NCORES)), trace=trace
    )
    LAST_RESULTS = res
    y = np.zeros((B, T, DIM), dtype=np.float32)
    for c in range(NCORES):
        y[c // 4] += res.results[c]["y"]
    return y


# revision 31
# speedup vs baseline: 1.0196x; 1.0030x over previous
"""Causal self-attention (B=2, T=2048, dim=2048, H=16, D=128) on 8 trn2 NeuronCores.

Sharding: data-parallel over batch (2 groups of 4 cores), tensor-parallel over
heads within a group (4 heads/core).  Each core computes its heads' QKV
projection (x @ Wqkv_part^T), RoPE, causal attention, and a partial output
projection against its W_proj column block; the host sums the 4 partials per
batch element.

v8 schedule (all matmul operands bf16, fp32 accumulation): one readiness-aware
software pipeline — attention units of window w are woven between the QKV
matmul sets of the SAME window and the proj units of window w-1, so the
ScalarE exp stream always drains under QKV/proj PE work:
  - all DRAM operands host-packed so every DMA lands in >=4KB contiguous
    per-partition lines; startup loads split across the two HWDGE queues
    (sync + scalar) in first-use order; first weight slab split so the
    first matmul gates on 128KB; rope tables shipped as 64 rows and
    mirrored on-chip (rows 0-63 == rows 64-127 up to the rotate sign).
  - RoPE fused into the QKV PSUM evacuation on DVE: rot halves are
    partition-offset multiplies against the sign-folded sin table, then
    dst = ps*cos + rot.  No PE swap matmul, no separate evacuation cast.
  - scores pipelined depth-3: PE score matmul -> ScalarE exp (bf16) -> PE
    PV.  Each head-window's pt tiles chain-accumulate into one fp16 tile
    on DVE (one add per unit), so the PE rowsum is a single 512-col
    all-ones matmul per head (denominators replicated across partitions).
  - softmax normalization off the PE: reciprocal_approx_fast + multiply (DVE).
  - QKV/proj PSUM evacuation on DVE/ScalarE; y DMA'd per window.
"""

import os

import numpy as np
import ml_dtypes

import concourse.bass as bass
import concourse.bacc as bacc
import concourse.tile as tile
import concourse.mybir as mybir
from concourse import bass_utils

BF16 = mybir.dt.bfloat16
F16 = mybir.dt.float16
F32 = mybir.dt.float32

B, T, DIM = 2, 2048, 2048
H, D = 16, 128
HL = 4                   # heads per core
NCORES = 8
NCHUNK = DIM // 128      # 16 contraction chunks
NW = T // 512            # 4 query windows
NTT = T // 128           # 16 token tiles
SCALE = 1.0 / float(np.sqrt(D))

_CACHE = {}
LAST_RESULTS = None


def _build_module():
    nc = bacc.Bacc("TRN2", target_bir_lowering=False, debug=False)
    # host-packed layouts: partition dim first, contiguous per-partition lines
    xPd = nc.dram_tensor("xP", (128, NW, NCHUNK, 512), BF16, kind="ExternalInput")
    wqPd = nc.dram_tensor("wqP", (128, 8, NCHUNK, 128), BF16, kind="ExternalInput")
    wvPd = nc.dram_tensor("wvP", (128, NCHUNK, 512), BF16, kind="ExternalInput")
    wpPd = nc.dram_tensor("wpP", (128, HL, DIM), BF16, kind="ExternalInput")
    cosT = nc.dram_tensor("cosT", (64, T), BF16, kind="ExternalInput")
    sinT = nc.dram_tensor("sinT", (64, T), BF16, kind="ExternalInput")
    tri = nc.dram_tensor("tri", (128, 128), BF16, kind="ExternalInput")
    ones = nc.dram_tensor("ones", (128, 128), F16, kind="ExternalInput")
    y = nc.dram_tensor("y", (T, DIM), F32, kind="ExternalOutput")

    Exp = mybir.ActivationFunctionType.Exp

    with tile.TileContext(nc) as tc:
        with (
            tc.tile_pool(name="const", bufs=1) as cpool,
            tc.tile_pool(name="xp", bufs=2) as xpool,
            tc.tile_pool(name="rotp", bufs=3) as rotpool,
            tc.tile_pool(name="ptp", bufs=6) as ptpool,
            tc.tile_pool(name="rsap", bufs=2) as rsapool,
            tc.tile_pool(name="rcpp", bufs=2) as rcppool,
            tc.tile_pool(name="yp", bufs=3) as ypool,
            tc.tile_pool(name="yp2", bufs=3) as ypool2,
            tc.tile_pool(name="psA", bufs=2, space="PSUM") as psA,
            tc.tile_pool(name="psS", bufs=3, space="PSUM") as psS,
            tc.tile_pool(name="psO", bufs=2, space="PSUM") as psO,
            tc.tile_pool(name="psR", bufs=1, space="PSUM") as psR,
        ):
            # per-set weight slabs: dependency granularity is per-tile, so a
            # qk set only waits for its own slab's DMA; slab 0 split so the
            # first matmul gates on a 128KB transfer
            wq0a = cpool.tile([128, 2, 128], BF16, tag="wq0a", name="wq0a")
            wq0b = cpool.tile([128, 14, 128], BF16, tag="wq0b", name="wq0b")
            wq_slab = [None] + [
                cpool.tile([128, NCHUNK, 128], BF16, tag=f"wq{s}", name=f"wq{s}")
                for s in range(1, 8)
            ]
            wv_sb = cpool.tile([128, NCHUNK, 512], BF16, tag="wv")
            wp_sb = cpool.tile([128, HL, DIM], BF16, tag="wp")
            cos_sb = cpool.tile([128, T], BF16, tag="cos")
            sin_sb = cpool.tile([128, T], BF16, tag="sin")
            tri_sb = cpool.tile([128, 128], BF16, tag="tri")
            ones_sb = cpool.tile([128, 128], F16, tag="ones")
            q_sb = cpool.tile([128, HL * T], BF16, tag="q")
            k_sb = cpool.tile([128, HL * T], BF16, tag="k")
            v_sb = cpool.tile([128, NTT * HL * D], BF16, tag="v")
            o_sb = cpool.tile([128, HL * T], BF16, tag="o")

            xbs = {}

            def load_xb(w):
                xb = xpool.tile([128, NCHUNK, 512], BF16, tag="xb", name="xb")
                nc.sync.dma_start(xb[:], xPd[:, w])
                xbs[w] = xb

            # Startup DMAs, first-needed first, split across both HWDGE
            # queues (sync + scalar); xb0 in 2-chunk pieces alternating
            # queues so set-0's chunk loop rarely outruns arrival; half-height
            # rope tables on the gpsimd SWDGE, mirrored on-chip by DVE.
            xb0 = xpool.tile([128, NCHUNK, 512], BF16, tag="xb", name="xb0")
            xbs[0] = xb0
            nc.scalar.dma_start(wq0a[:], wqPd[:, 0, 0:2, :])
            nc.sync.dma_start(xb0[:, 0:1, :], xPd[:, 0, 0:1, :])
            nc.gpsimd.dma_start(cos_sb[0:64, :], cosT[:, :])
            nc.gpsimd.dma_start(sin_sb[64:128, :], sinT[:, :])
            nc.scalar.dma_start(wq0b[:], wqPd[:, 0, 2:16, :])
            nc.sync.dma_start(xb0[:, 1:3, :], xPd[:, 0, 1:3, :])
            nc.sync.dma_start(xb0[:, 3:6, :], xPd[:, 0, 3:6, :])
            nc.scalar.dma_start(xb0[:, 6:8, :], xPd[:, 0, 6:8, :])
            nc.sync.dma_start(xb0[:, 8:10, :], xPd[:, 0, 8:10, :])
            nc.scalar.dma_start(wq_slab[1][:], wqPd[:, 1])
            nc.sync.dma_start(xb0[:, 10:12, :], xPd[:, 0, 10:12, :])
            nc.scalar.dma_start(xb0[:, 12:14, :], xPd[:, 0, 12:14, :])
            nc.sync.dma_start(xb0[:, 14:16, :], xPd[:, 0, 14:16, :])
            nc.gpsimd.dma_start(tri_sb[:], tri[:, :])
            nc.gpsimd.dma_start(ones_sb[:], ones[:, :])
            nc.scalar.dma_start(wq_slab[2][:], wqPd[:, 2])
            nc.sync.dma_start(wq_slab[3][:], wqPd[:, 3])
            nc.sync.dma_start(wv_sb[:, 0:8, :], wvPd[:, 0:8, :])
            nc.scalar.dma_start(wv_sb[:, 8:16, :], wvPd[:, 8:16, :])
            nc.sync.dma_start(wq_slab[4][:], wqPd[:, 4])
            nc.scalar.dma_start(wq_slab[5][:], wqPd[:, 5])
            nc.sync.dma_start(wq_slab[6][:], wqPd[:, 6])
            nc.scalar.dma_start(wq_slab[7][:], wqPd[:, 7])
            # mirror the half-height tables on DVE (cos rows repeat; sin rows
            # 0-63 carry the folded rotate-half sign)
            nc.vector.tensor_copy(cos_sb[64:128, :], cos_sb[0:64, :])
            nc.vector.tensor_scalar_mul(sin_sb[0:64, :], sin_sb[64:128, :], -1.0)

            def qkv_bigs(w):
                """[('q'|'k'|'v', idx, closure)] — 8 qk sets then 4 v sets."""
                wsl = slice(w * 512, (w + 1) * 512)

                def qk_set(grp, j):
                    def run():
                        xb = xbs[w]
                        dst = q_sb if grp == 0 else k_sb
                        ps = psA.tile([128, 512], F32, tag="ps", name="ps")
                        s = grp * 4 + j
                        for c in range(NCHUNK):
                            if s == 0:
                                slab = wq0a[:, c, :] if c < 2 else wq0b[:, c - 2, :]
                            else:
                                slab = wq_slab[s][:, c, :]
                            nc.tensor.matmul(
                                ps[:],
                                slab,
                                xb[:, c, :],
                                start=(c == 0),
                                stop=(c == NCHUNK - 1),
                            )
                        # fused RoPE evacuation: dst = ps*cos + rot(ps)*sin
                        sl = slice(j * T + w * 512, j * T + (w + 1) * 512)
                        rot = rotpool.tile([128, 512], BF16, tag="rot", name="rot")
                        nc.vector.tensor_mul(
                            rot[0:64, :], ps[64:128, :], sin_sb[0:64, wsl]
                        )
                        nc.vector.tensor_mul(
                            rot[64:128, :], ps[0:64, :], sin_sb[64:128, wsl]
                        )
                        nc.vector.tensor_mul(dst[:, sl], ps[:], cos_sb[:, wsl])
                        nc.vector.tensor_add(dst[:, sl], dst[:, sl], rot[:])

                    return run

                def v_set(ttl):
                    def run():
                        xb = xbs[w]
                        ttg = w * 4 + ttl
                        ps = psA.tile([128, 512], F32, tag="ps", name="ps")
                        for c in range(NCHUNK):
                            nc.tensor.matmul(
                                ps[:],
                                xb[:, c, ttl * 128 : (ttl + 1) * 128],
                                wv_sb[:, c, :],
                                start=(c == 0),
                                stop=(c == NCHUNK - 1),
                            )
                        nc.scalar.copy(v_sb[:, ttg * 512 : (ttg + 1) * 512], ps[:])

                    return run

                return (
                    [("q", j, qk_set(0, j)) for j in range(HL)]
                    + [("v", ttl, v_set(ttl)) for ttl in range(4)]
                    + [("k", j, qk_set(1, j)) for j in range(HL)]
                )

            def attn_fillers(w, pos):
                """Per-(head, key-tile) closures + their readiness (number of
                this window's qkv bigs that must have been emitted first).
                All of a head-window's pt tiles chain-accumulate into one
                fp16 acc on DVE (one add per unit); a single 512-col
                all-ones matmul per head produces the softmax denominators,
                emitted one unit into the next head so the single-bank
                rowsum tile's WAR on the previous reciprocal is covered."""
                nkt = 4 * w + 4
                OD = 4 * w            # off-diagonal tiles per head
                nu = HL * nkt
                state = {
                    "pend": [],
                    "issued": 0,
                    "oT": {},
                    "pt0": None,
                    "acc": {},
                }

                def geom(kt):
                    if kt < OD:
                        return 512 * w, 512, False
                    q0 = 128 * kt
                    return q0, 512 * (w + 1) - 128 * kt, True

                def issue_score(u):
                    h, kt = divmod(u, nkt)
                    hq = h * T
                    q0, n, diag = geom(kt)
                    st = psS.tile([128, 512], F32, tag="st", name="st")
                    nc.tensor.matmul(
                        st[:, :n],
                        k_sb[:, hq + kt * 128 : hq + (kt + 1) * 128],
                        q_sb[:, hq + q0 : hq + q0 + n],
                        start=True,
                        stop=True,
                    )
                    pt = ptpool.tile([128, 512], BF16, tag="pt", name="pt")
                    nc.scalar.activation(
                        pt[:, :n], st[:, :n], Exp, bias=0.0, scale=SCALE
                    )
                    if diag:
                        nc.vector.tensor_mul(pt[:, 0:128], pt[:, 0:128], tri_sb[:])
                    return pt, q0, n

                DEPTH = 3

                def flush_rs(h):
                    # single rowsum matmul for head h, then the off-PE
                    # normalization chain
                    rs = psR.tile([128, 512], F32, tag="rs", name="rs")
                    nc.tensor.matmul(
                        rs[:],
                        ones_sb[:],
                        state["acc"].pop(h)[:],
                        start=True,
                        stop=True,
                    )
                    hq = h * T
                    rcp = rcppool.tile([128, 512], F32, tag="rcp", name="rcp")
                    nc.vector.reciprocal_approx_fast(rcp[:], rs[:])
                    nc.vector.tensor_mul(
                        o_sb[:, hq + w * 512 : hq + (w + 1) * 512],
                        state["oT"].pop(h)[:],
                        rcp[:],
                    )

                def unit(u):
                    def run():
                        h, kt = divmod(u, nkt)
                        while state["issued"] < min(u + DEPTH + 1, nu):
                            state["pend"].append(issue_score(state["issued"]))
                            state["issued"] += 1
                        if kt == 0:
                            state["oT"][h] = psO.tile(
                                [128, 512], F32, tag="oT", name="oT"
                            )
                        pt, q0, n = state["pend"].pop(0)
                        off = q0 - 512 * w
                        nc.tensor.matmul(
                            state["oT"][h][:, off:512],
                            v_sb[:, kt * 512 + h * 128 : kt * 512 + (h + 1) * 128],
                            pt[:, :n],
                            start=(kt == 0),
                            stop=(kt == nkt - 1),
                        )
                        # denominator accumulation (kt = 0 always has n = 512)
                        if kt == 0:
                            state["pt0"] = pt
                        elif kt == 1:
                            acc = rsapool.tile(
                                [128, 512], F16, tag="acc", name="acc"
                            )
                            if n == 512:
                                nc.vector.tensor_add(
                                    acc[:], state["pt0"][:], pt[:]
                                )
                            else:
                                nc.vector.tensor_copy(
                                    acc[:, 0:off], state["pt0"][:, 0:off]
                                )
                                nc.vector.tensor_add(
                                    acc[:, off:512],
                                    state["pt0"][:, off:512],
                                    pt[:, :n],
                                )
                            state["acc"][h] = acc
                        else:
                            acc = state["acc"][h]
                            nc.vector.tensor_add(
                                acc[:, off:512], acc[:, off:512], pt[:, :n]
                            )
                        if kt == 1 and h > 0:
                            flush_rs(h - 1)
                        if u == nu - 1:
                            flush_rs(h)

                    return run

                def score_ra(u):
                    h, kt = divmod(u, nkt)
                    if kt < OD:
                        return pos[("q", h)] + 1
                    return pos[("k", h)] + 1

                def pv_ra(u):
                    h, kt = divmod(u, nkt)
                    if kt < OD:
                        return 0
                    return pos[("v", kt - OD)] + 1

                fillers = []
                ready = []
                for u in range(nu):
                    ra = pv_ra(u)
                    for uu in range(u, min(u + DEPTH + 1, nu)):
                        ra = max(ra, score_ra(uu))
                    fillers.append(unit(u))
                    ready.append(ra)
                return fillers, ready

            def proj_bigs(w, pools=None, alt_dma=False):
                out = []

                def unit(tt, nwi, pool, ptag, dve_evac, dmae):
                    def run():
                        yps = pool.tile([128, 512], F32, tag=ptag, name="yps")
                        for hh in range(HL):
                            nc.tensor.matmul(
                                yps[:],
                                o_sb[:, hh * T + tt * 128 : hh * T + (tt + 1) * 128],
                                wp_sb[:, hh, nwi * 512 : (nwi + 1) * 512],
                                start=(hh == 0),
                                stop=(hh == HL - 1),
                            )
                        ysb = ypool.tile([128, 512], F32, tag="ysb", name="ysb")
                        if dve_evac:
                            nc.vector.tensor_copy(ysb[:], yps[:])
                        else:
                            nc.scalar.copy(ysb[:], yps[:])
                        dmae.dma_start(
                            y[tt * 128 : (tt + 1) * 128, nwi * 512 : (nwi + 1) * 512],
                            ysb[:],
                        )

                    return run

                i = 0
                for tt in range(4 * w, 4 * w + 4):
                    for nwi in range(DIM // 512):
                        if pools:
                            pool, ptag = pools[i % len(pools)]
                        else:
                            pool, ptag = psA, "ps"
                        # in the final batch (attention over) split the
                        # evacuations across ACT and DVE and put each unit's
                        # DMA config on the opposite sequencer so neither
                        # sequencer serializes copy + trigger
                        dve = alt_dma and (i % 2 == 1)
                        dmae = nc.scalar if dve else nc.sync
                        out.append(
                            ("p", i, unit(tt, nwi, pool, ptag, dve, dmae))
                        )
                        i += 1
                return out

            def weave(bigs, fillers, ready):
                nb = len(bigs)
                nf = len(fillers)
                done = 0
                for i, (_, _, b) in enumerate(bigs):
                    b()
                    # cap the per-big burst so late-ready units don't flood
                    # the exp engine all at once
                    target = min(int(round(nf * (i + 1) / nb)), done + 4)
                    while done < nf and done < target and ready[done] <= i + 1:
                        fillers[done]()
                        done += 1
                while done < nf:
                    fillers[done]()
                    done += 1

            for w in range(NW):
                bigs = qkv_bigs(w)
                if w + 1 < NW:
                    # prefetch next window's x once the startup crunch is over
                    bigs.insert(6, ("x", w + 1, lambda wn=w + 1: load_xb(wn)))
                if w == 0:
                    bigs.append(
                        ("wp", 0, lambda: nc.sync.dma_start(wp_sb[:], wpPd[:]))
                    )
                if w >= 1:
                    bigs += proj_bigs(w - 1)
                pos = {(lbl, idx): i for i, (lbl, idx, _) in enumerate(bigs)}
                fillers, ready = attn_fillers(w, pos)
                weave(bigs, fillers, ready)
            # after the last window's attention, all PSUM pools are free:
            # cycle the final proj units across them so the evacuation chain
            # never blocks the PE on a bank WAR
            # final batch: pair adjacent nwi units into one [128,1024] ysb
            # tile - one DMA per pair, copies split across ACT and DVE,
            # pair DMAs alternating both HWDGE queues (exp stream is over)
            def final_pair(tt, nwi0, dmae):
                ysb2 = ypool2.tile([128, 1024], F32, tag="ysb2", name="ysb2")
                for half in range(2):
                    nwi = nwi0 + half
                    # half 0 in psA, half 1 in psS (free after attention):
                    # the next pair's bank is then recycled a full pair
                    # earlier, so its first matmul never waits on the
                    # previous pair's trailing evacuation copy
                    if half == 0:
                        yps = psA.tile([128, 512], F32, tag="ps", name="yps")
                    else:
                        yps = psS.tile([128, 512], F32, tag="st", name="yps")
                    for hh in range(HL):
                        nc.tensor.matmul(
                            yps[:],
                            o_sb[:, hh * T + tt * 128 : hh * T + (tt + 1) * 128],
                            wp_sb[:, hh, nwi * 512 : (nwi + 1) * 512],
                            start=(hh == 0),
                            stop=(hh == HL - 1),
                        )
                    if half == 0:
                        nc.scalar.copy(ysb2[:, 0:512], yps[:])
                    else:
                        nc.vector.tensor_copy(ysb2[:, 512:1024], yps[:])
                dmae.dma_start(
                    y[tt * 128 : (tt + 1) * 128, nwi0 * 512 : (nwi0 + 2) * 512],
                    ysb2[:],
                )

            pi = 0
            for tt in range(4 * (NW - 1), 4 * NW):
                for nwi0 in (0, 2):
                    if tt == 4 * NW - 1 and nwi0 == 2:
                        break
                    final_pair(tt, nwi0, nc.scalar if pi % 2 else nc.sync)
                    pi += 1
            # final unit split into column halves on both HWDGE queues so the
            # exposed tail evacuation + y DMA is halved
            tt = NTT - 1
            r0 = slice(tt * 128, (tt + 1) * 128)
            for nwi, half in ((2, 0), (3, 1)):
                yps = psA.tile([128, 512], F32, tag="ps", name="yps")
                for hh in range(HL):
                    nc.tensor.matmul(
                        yps[:],
                        o_sb[:, hh * T + tt * 128 : hh * T + (tt + 1) * 128],
                        wp_sb[:, hh, nwi * 512 : (nwi + 1) * 512],
                        start=(hh == 0),
                        stop=(hh == HL - 1),
                    )
                ysb = ypool.tile([128, 512], F32, tag="ysb", name="ysb")
                if half == 0:
                    nc.vector.tensor_copy(ysb[:], yps[:])
                    nc.sync.dma_start(
                        y[r0, nwi * 512 : (nwi + 1) * 512], ysb[:]
                    )
                else:
                    nc.scalar.copy(ysb[:, 0:256], yps[:, 0:256])
                    nc.scalar.dma_start(
                        y[r0, nwi * 512 : nwi * 512 + 256], ysb[:, 0:256]
                    )
                    nc.vector.tensor_copy(ysb[:, 256:512], yps[:, 256:512])
                    nc.sync.dma_start(
                        y[r0, nwi * 512 + 256 : (nwi + 1) * 512], ysb[:, 256:512]
                    )

    nc.compile()
    return nc


def _rope_tables():
    inv_freq = (
        1.0 / (10000.0 ** (np.arange(0, D, 2, dtype=np.float32) / np.float32(D)))
    ).astype(np.float32)
    tpos = np.arange(T, dtype=np.float32)
    freqs = tpos[:, None] * inv_freq[None, :]  # (T, 64)
    cosT = np.ascontiguousarray(np.cos(freqs).T)  # (64, T)
    sinT = np.ascontiguousarray(np.sin(freqs).T)
    return (
        cosT.astype(ml_dtypes.bfloat16),
        sinT.astype(ml_dtypes.bfloat16),
    )


def make_in_maps(x, W_qkv, W_proj):
    cosT, sinT = _rope_tables()
    tri = (np.arange(128)[None, :] >= np.arange(128)[:, None]).astype(
        ml_dtypes.bfloat16
    )
    tri = np.ascontiguousarray(tri)
    ones = np.ones((128, 128), dtype=np.float16)

    xPs = {}
    for b in range(B):
        xt = np.ascontiguousarray(x[b].T).astype(ml_dtypes.bfloat16)  # (dim, T)
        xPs[b] = np.ascontiguousarray(
            xt.reshape(NCHUNK, 128, NW, 512).transpose(1, 2, 0, 3)
        )

    wqPs, wvPs, wpPs = {}, {}, {}
    for g in range(4):
        Wq = W_qkv[512 * g : 512 * (g + 1)]
        Wk = W_qkv[2048 + 512 * g : 2048 + 512 * (g + 1)]
        Wv = W_qkv[4096 + 512 * g : 4096 + 512 * (g + 1)]
        Wc = np.concatenate([Wq, Wk, Wv], axis=0)  # (1536, 2048)
        A = (
            np.ascontiguousarray(Wc.T)
            .astype(ml_dtypes.bfloat16)
            .reshape(NCHUNK, 128, 1536)
            .transpose(1, 0, 2)
        )  # [p, c, e]
        wqPs[g] = np.ascontiguousarray(
            A[:, :, :1024].reshape(128, NCHUNK, 8, 128).transpose(0, 2, 1, 3)
        )  # [p, s, c, e]
        wvPs[g] = np.ascontiguousarray(A[:, :, 1024:])  # [p, c, e512]
        wpPs[g] = np.ascontiguousarray(
            np.ascontiguousarray(W_proj[:, 512 * g : 512 * (g + 1)].T)
            .astype(ml_dtypes.bfloat16)
            .reshape(HL, 128, DIM)
            .transpose(1, 0, 2)
        )  # [p, h, n]

    in_maps = []
    for c in range(NCORES):
        b, g = divmod(c, 4)
        in_maps.append(
            {
                "xP": xPs[b],
                "wqP": wqPs[g],
                "wvP": wvPs[g],
                "wpP": wpPs[g],
                "cosT": cosT,
                "sinT": sinT,
                "tri": tri,
                "ones": ones,
            }
        )
    return in_maps


def kernel(x, W_qkv, W_proj):
    global LAST_RESULTS
    x = np.asarray(x, dtype=np.float32)
    W_qkv = np.asarray(W_qkv, dtype=np.float32)
    W_proj = np.asarray(W_proj, dtype=np.float32)
    assert x.shape == (B, T, DIM) and W_qkv.shape == (3 * H * D, DIM)

    if "nc" not in _CACHE:
        _CACHE["nc"] = _build_module()
    nc = _CACHE["nc"]

    in_maps = make_in_maps(x, W_qkv, W_proj)
    trace = os.environ.get("KERNEL_TRACE", "0") == "1"
    res = bass_utils.run_bass_kernel_spmd(
        nc, in_maps, core_ids=list(range(NCORES)), trace=trace
    )
    LAST_RESULTS = res
    y = np.zeros((B, T, DIM), dtype=np.float32)
    for c in range(NCORES):
        y[c // 4] += res.results[c]["y"]
    return y


# revision 32
# speedup vs baseline: 1.0236x; 1.0039x over previous
"""Causal self-attention (B=2, T=2048, dim=2048, H=16, D=128) on 8 trn2 NeuronCores.

Sharding: data-parallel over batch (2 groups of 4 cores), tensor-parallel over
heads within a group (4 heads/core).  Each core computes its heads' QKV
projection (x @ Wqkv_part^T), RoPE, causal attention, and a partial output
projection against its W_proj column block; the host sums the 4 partials per
batch element.

v8 schedule (all matmul operands bf16, fp32 accumulation): one readiness-aware
software pipeline — attention units of window w are woven between the QKV
matmul sets of the SAME window and the proj units of window w-1, so the
ScalarE exp stream always drains under QKV/proj PE work:
  - all DRAM operands host-packed so every DMA lands in >=4KB contiguous
    per-partition lines; startup loads split across the two HWDGE queues
    (sync + scalar) in first-use order; first weight slab split so the
    first matmul gates on 128KB; rope tables shipped as 64 rows and
    mirrored on-chip (rows 0-63 == rows 64-127 up to the rotate sign).
  - RoPE fused into the QKV PSUM evacuation on DVE: rot halves are
    partition-offset multiplies against the sign-folded sin table, then
    dst = ps*cos + rot.  No PE swap matmul, no separate evacuation cast.
  - scores pipelined depth-3: PE score matmul -> ScalarE exp (bf16) -> PE
    PV.  Each head-window's pt tiles chain-accumulate into one fp16 tile
    on DVE (one add per unit), so the PE rowsum is a single 512-col
    all-ones matmul per head (denominators replicated across partitions).
  - softmax normalization off the PE: reciprocal_approx_fast + multiply (DVE).
  - QKV/proj PSUM evacuation on DVE/ScalarE; y DMA'd per window.
"""

import os

import numpy as np
import ml_dtypes

import concourse.bass as bass
import concourse.bacc as bacc
import concourse.tile as tile
import concourse.mybir as mybir
from concourse import bass_utils

BF16 = mybir.dt.bfloat16
F16 = mybir.dt.float16
F32 = mybir.dt.float32

B, T, DIM = 2, 2048, 2048
H, D = 16, 128
HL = 4                   # heads per core
NCORES = 8
NCHUNK = DIM // 128      # 16 contraction chunks
NW = T // 512            # 4 query windows
NTT = T // 128           # 16 token tiles
SCALE = 1.0 / float(np.sqrt(D))

_CACHE = {}
LAST_RESULTS = None


def _build_module():
    nc = bacc.Bacc("TRN2", target_bir_lowering=False, debug=False)
    # host-packed layouts: partition dim first, contiguous per-partition lines
    xPd = nc.dram_tensor("xP", (128, NW, NCHUNK, 512), BF16, kind="ExternalInput")
    wqPd = nc.dram_tensor("wqP", (128, 8, NCHUNK, 128), BF16, kind="ExternalInput")
    wvPd = nc.dram_tensor("wvP", (128, NCHUNK, 512), BF16, kind="ExternalInput")
    wpPd = nc.dram_tensor("wpP", (128, HL, DIM), BF16, kind="ExternalInput")
    cosT = nc.dram_tensor("cosT", (64, T), BF16, kind="ExternalInput")
    sinT = nc.dram_tensor("sinT", (64, T), BF16, kind="ExternalInput")
    tri = nc.dram_tensor("tri", (128, 128), BF16, kind="ExternalInput")
    ones = nc.dram_tensor("ones", (128, 128), F16, kind="ExternalInput")
    y = nc.dram_tensor("y", (T, DIM), F32, kind="ExternalOutput")

    Exp = mybir.ActivationFunctionType.Exp

    with tile.TileContext(nc) as tc:
        with (
            tc.tile_pool(name="const", bufs=1) as cpool,
            tc.tile_pool(name="xp", bufs=2) as xpool,
            tc.tile_pool(name="rotp", bufs=3) as rotpool,
            tc.tile_pool(name="ptp", bufs=6) as ptpool,
            tc.tile_pool(name="rsap", bufs=2) as rsapool,
            tc.tile_pool(name="rcpp", bufs=2) as rcppool,
            tc.tile_pool(name="yp", bufs=6) as ypool,
            tc.tile_pool(name="yp2", bufs=3) as ypool2,
            tc.tile_pool(name="psA", bufs=2, space="PSUM") as psA,
            tc.tile_pool(name="psS", bufs=3, space="PSUM") as psS,
            tc.tile_pool(name="psO", bufs=2, space="PSUM") as psO,
            tc.tile_pool(name="psR", bufs=1, space="PSUM") as psR,
        ):
            # per-set weight slabs: dependency granularity is per-tile, so a
            # qk set only waits for its own slab's DMA; slab 0 split so the
            # first matmul gates on a 128KB transfer
            wq0a = cpool.tile([128, 2, 128], BF16, tag="wq0a", name="wq0a")
            wq0b = cpool.tile([128, 14, 128], BF16, tag="wq0b", name="wq0b")
            wq_slab = [None] + [
                cpool.tile([128, NCHUNK, 128], BF16, tag=f"wq{s}", name=f"wq{s}")
                for s in range(1, 8)
            ]
            wv_sb = cpool.tile([128, NCHUNK, 512], BF16, tag="wv")
            wp_sb = cpool.tile([128, HL, DIM], BF16, tag="wp")
            cos_sb = cpool.tile([128, T], BF16, tag="cos")
            sin_sb = cpool.tile([128, T], BF16, tag="sin")
            tri_sb = cpool.tile([128, 128], BF16, tag="tri")
            ones_sb = cpool.tile([128, 128], F16, tag="ones")
            q_sb = cpool.tile([128, HL * T], BF16, tag="q")
            k_sb = cpool.tile([128, HL * T], BF16, tag="k")
            v_sb = cpool.tile([128, NTT * HL * D], BF16, tag="v")
            o_sb = cpool.tile([128, HL * T], BF16, tag="o")

            xbs = {}

            def load_xb(w):
                xb = xpool.tile([128, NCHUNK, 512], BF16, tag="xb", name="xb")
                nc.sync.dma_start(xb[:], xPd[:, w])
                xbs[w] = xb

            # Startup DMAs, first-needed first, split across both HWDGE
            # queues (sync + scalar); xb0 in 2-chunk pieces alternating
            # queues so set-0's chunk loop rarely outruns arrival; half-height
            # rope tables on the gpsimd SWDGE, mirrored on-chip by DVE.
            xb0 = xpool.tile([128, NCHUNK, 512], BF16, tag="xb", name="xb0")
            xbs[0] = xb0
            nc.scalar.dma_start(wq0a[:], wqPd[:, 0, 0:2, :])
            nc.sync.dma_start(xb0[:, 0:1, :], xPd[:, 0, 0:1, :])
            nc.gpsimd.dma_start(cos_sb[0:64, :], cosT[:, :])
            nc.gpsimd.dma_start(sin_sb[64:128, :], sinT[:, :])
            nc.scalar.dma_start(wq0b[:], wqPd[:, 0, 2:16, :])
            nc.sync.dma_start(xb0[:, 1:3, :], xPd[:, 0, 1:3, :])
            nc.sync.dma_start(xb0[:, 3:6, :], xPd[:, 0, 3:6, :])
            nc.scalar.dma_start(xb0[:, 6:8, :], xPd[:, 0, 6:8, :])
            nc.sync.dma_start(xb0[:, 8:10, :], xPd[:, 0, 8:10, :])
            nc.scalar.dma_start(wq_slab[1][:], wqPd[:, 1])
            nc.sync.dma_start(xb0[:, 10:12, :], xPd[:, 0, 10:12, :])
            nc.scalar.dma_start(xb0[:, 12:14, :], xPd[:, 0, 12:14, :])
            nc.sync.dma_start(xb0[:, 14:16, :], xPd[:, 0, 14:16, :])
            nc.gpsimd.dma_start(tri_sb[:], tri[:, :])
            nc.gpsimd.dma_start(ones_sb[:], ones[:, :])
            nc.scalar.dma_start(wq_slab[2][:], wqPd[:, 2])
            nc.sync.dma_start(wq_slab[3][:], wqPd[:, 3])
            nc.sync.dma_start(wv_sb[:, 0:8, :], wvPd[:, 0:8, :])
            nc.scalar.dma_start(wv_sb[:, 8:16, :], wvPd[:, 8:16, :])
            nc.sync.dma_start(wq_slab[4][:], wqPd[:, 4])
            nc.scalar.dma_start(wq_slab[5][:], wqPd[:, 5])
            nc.sync.dma_start(wq_slab[6][:], wqPd[:, 6])
            nc.scalar.dma_start(wq_slab[7][:], wqPd[:, 7])
            # mirror the half-height tables on DVE (cos rows repeat; sin rows
            # 0-63 carry the folded rotate-half sign)
            nc.vector.tensor_copy(cos_sb[64:128, :], cos_sb[0:64, :])
            nc.vector.tensor_scalar_mul(sin_sb[0:64, :], sin_sb[64:128, :], -1.0)

            def qkv_bigs(w):
                """[('q'|'k'|'v', idx, closure)] — 8 qk sets then 4 v sets."""
                wsl = slice(w * 512, (w + 1) * 512)

                def qk_set(grp, j):
                    def run():
                        xb = xbs[w]
                        dst = q_sb if grp == 0 else k_sb
                        ps = psA.tile([128, 512], F32, tag="ps", name="ps")
                        s = grp * 4 + j
                        for c in range(NCHUNK):
                            if s == 0:
                                slab = wq0a[:, c, :] if c < 2 else wq0b[:, c - 2, :]
                            else:
                                slab = wq_slab[s][:, c, :]
                            nc.tensor.matmul(
                                ps[:],
                                slab,
                                xb[:, c, :],
                                start=(c == 0),
                                stop=(c == NCHUNK - 1),
                            )
                        # fused RoPE evacuation: dst = ps*cos + rot(ps)*sin
                        sl = slice(j * T + w * 512, j * T + (w + 1) * 512)
                        rot = rotpool.tile([128, 512], BF16, tag="rot", name="rot")
                        nc.vector.tensor_mul(
                            rot[0:64, :], ps[64:128, :], sin_sb[0:64, wsl]
                        )
                        nc.vector.tensor_mul(
                            rot[64:128, :], ps[0:64, :], sin_sb[64:128, wsl]
                        )
                        nc.vector.tensor_mul(dst[:, sl], ps[:], cos_sb[:, wsl])
                        nc.vector.tensor_add(dst[:, sl], dst[:, sl], rot[:])

                    return run

                def v_set(ttl):
                    def run():
                        xb = xbs[w]
                        ttg = w * 4 + ttl
                        ps = psA.tile([128, 512], F32, tag="ps", name="ps")
                        for c in range(NCHUNK):
                            nc.tensor.matmul(
                                ps[:],
                                xb[:, c, ttl * 128 : (ttl + 1) * 128],
                                wv_sb[:, c, :],
                                start=(c == 0),
                                stop=(c == NCHUNK - 1),
                            )
                        nc.scalar.copy(v_sb[:, ttg * 512 : (ttg + 1) * 512], ps[:])

                    return run

                return (
                    [("q", j, qk_set(0, j)) for j in range(HL)]
                    + [("v", ttl, v_set(ttl)) for ttl in range(4)]
                    + [("k", j, qk_set(1, j)) for j in range(HL)]
                )

            def attn_fillers(w, pos):
                """Per-(head, key-tile) closures + their readiness (number of
                this window's qkv bigs that must have been emitted first).
                All of a head-window's pt tiles chain-accumulate into one
                fp16 acc on DVE (one add per unit); a single 512-col
                all-ones matmul per head produces the softmax denominators,
                emitted one unit into the next head so the single-bank
                rowsum tile's WAR on the previous reciprocal is covered."""
                nkt = 4 * w + 4
                OD = 4 * w            # off-diagonal tiles per head
                nu = HL * nkt
                state = {
                    "pend": [],
                    "issued": 0,
                    "oT": {},
                    "pt0": None,
                    "acc": {},
                }

                def geom(kt):
                    if kt < OD:
                        return 512 * w, 512, False
                    q0 = 128 * kt
                    return q0, 512 * (w + 1) - 128 * kt, True

                def issue_score(u):
                    h, kt = divmod(u, nkt)
                    hq = h * T
                    q0, n, diag = geom(kt)
                    st = psS.tile([128, 512], F32, tag="st", name="st")
                    nc.tensor.matmul(
                        st[:, :n],
                        k_sb[:, hq + kt * 128 : hq + (kt + 1) * 128],
                        q_sb[:, hq + q0 : hq + q0 + n],
                        start=True,
                        stop=True,
                    )
                    pt = ptpool.tile([128, 512], BF16, tag="pt", name="pt")
                    nc.scalar.activation(
                        pt[:, :n], st[:, :n], Exp, bias=0.0, scale=SCALE
                    )
                    if diag:
                        nc.vector.tensor_mul(pt[:, 0:128], pt[:, 0:128], tri_sb[:])
                    return pt, q0, n

                DEPTH = 3

                def flush_rs(h):
                    # single rowsum matmul for head h, then the off-PE
                    # normalization chain
                    rs = psR.tile([128, 512], F32, tag="rs", name="rs")
                    nc.tensor.matmul(
                        rs[:],
                        ones_sb[:],
                        state["acc"].pop(h)[:],
                        start=True,
                        stop=True,
                    )
                    hq = h * T
                    rcp = rcppool.tile([128, 512], F32, tag="rcp", name="rcp")
                    nc.vector.reciprocal_approx_fast(rcp[:], rs[:])
                    nc.vector.tensor_mul(
                        o_sb[:, hq + w * 512 : hq + (w + 1) * 512],
                        state["oT"].pop(h)[:],
                        rcp[:],
                    )

                def unit(u):
                    def run():
                        h, kt = divmod(u, nkt)
                        while state["issued"] < min(u + DEPTH + 1, nu):
                            state["pend"].append(issue_score(state["issued"]))
                            state["issued"] += 1
                        if kt == 0:
                            state["oT"][h] = psO.tile(
                                [128, 512], F32, tag="oT", name="oT"
                            )
                        pt, q0, n = state["pend"].pop(0)
                        off = q0 - 512 * w
                        nc.tensor.matmul(
                            state["oT"][h][:, off:512],
                            v_sb[:, kt * 512 + h * 128 : kt * 512 + (h + 1) * 128],
                            pt[:, :n],
                            start=(kt == 0),
                            stop=(kt == nkt - 1),
                        )
                        # denominator accumulation (kt = 0 always has n = 512)
                        if kt == 0:
                            state["pt0"] = pt
                        elif kt == 1:
                            acc = rsapool.tile(
                                [128, 512], F16, tag="acc", name="acc"
                            )
                            if n == 512:
                                nc.vector.tensor_add(
                                    acc[:], state["pt0"][:], pt[:]
                                )
                            else:
                                nc.vector.tensor_copy(
                                    acc[:, 0:off], state["pt0"][:, 0:off]
                                )
                                nc.vector.tensor_add(
                                    acc[:, off:512],
                                    state["pt0"][:, off:512],
                                    pt[:, :n],
                                )
                            state["acc"][h] = acc
                        else:
                            acc = state["acc"][h]
                            nc.vector.tensor_add(
                                acc[:, off:512], acc[:, off:512], pt[:, :n]
                            )
                        if kt == 1 and h > 0:
                            flush_rs(h - 1)
                        if u == nu - 1:
                            flush_rs(h)

                    return run

                def score_ra(u):
                    h, kt = divmod(u, nkt)
                    if kt < OD:
                        return pos[("q", h)] + 1
                    return pos[("k", h)] + 1

                def pv_ra(u):
                    h, kt = divmod(u, nkt)
                    if kt < OD:
                        return 0
                    return pos[("v", kt - OD)] + 1

                fillers = []
                ready = []
                for u in range(nu):
                    ra = pv_ra(u)
                    for uu in range(u, min(u + DEPTH + 1, nu)):
                        ra = max(ra, score_ra(uu))
                    fillers.append(unit(u))
                    ready.append(ra)
                return fillers, ready

            def proj_bigs(w, pools=None, alt_dma=False):
                out = []

                def unit(tt, nwi, pool, ptag, dve_evac, dmae):
                    def run():
                        yps = pool.tile([128, 512], F32, tag=ptag, name="yps")
                        for hh in range(HL):
                            nc.tensor.matmul(
                                yps[:],
                                o_sb[:, hh * T + tt * 128 : hh * T + (tt + 1) * 128],
                                wp_sb[:, hh, nwi * 512 : (nwi + 1) * 512],
                                start=(hh == 0),
                                stop=(hh == HL - 1),
                            )
                        ysb = ypool.tile([128, 512], F32, tag="ysb", name="ysb")
                        if dve_evac:
                            nc.vector.tensor_copy(ysb[:], yps[:])
                        else:
                            nc.scalar.copy(ysb[:], yps[:])
                        dmae.dma_start(
                            y[tt * 128 : (tt + 1) * 128, nwi * 512 : (nwi + 1) * 512],
                            ysb[:],
                        )

                    return run

                i = 0
                for tt in range(4 * w, 4 * w + 4):
                    for nwi in range(DIM // 512):
                        if pools:
                            pool, ptag = pools[i % len(pools)]
                        else:
                            pool, ptag = psA, "ps"
                        # in the final batch (attention over) split the
                        # evacuations across ACT and DVE and put each unit's
                        # DMA config on the opposite sequencer so neither
                        # sequencer serializes copy + trigger
                        dve = alt_dma and (i % 2 == 1)
                        dmae = nc.scalar if dve else nc.sync
                        out.append(
                            ("p", i, unit(tt, nwi, pool, ptag, dve, dmae))
                        )
                        i += 1
                return out

            def weave(bigs, fillers, ready):
                nb = len(bigs)
                nf = len(fillers)
                done = 0
                for i, (_, _, b) in enumerate(bigs):
                    b()
                    # cap the per-big burst so late-ready units don't flood
                    # the exp engine all at once
                    target = min(int(round(nf * (i + 1) / nb)), done + 4)
                    while done < nf and done < target and ready[done] <= i + 1:
                        fillers[done]()
                        done += 1
                while done < nf:
                    fillers[done]()
                    done += 1

            for w in range(NW):
                bigs = qkv_bigs(w)
                if w + 1 < NW:
                    # prefetch next window's x once the startup crunch is over
                    bigs.insert(6, ("x", w + 1, lambda wn=w + 1: load_xb(wn)))
                if w == 0:
                    bigs.append(
                        ("wp", 0, lambda: nc.sync.dma_start(wp_sb[:], wpPd[:]))
                    )
                if w >= 1:
                    bigs += proj_bigs(w - 1)
                pos = {(lbl, idx): i for i, (lbl, idx, _) in enumerate(bigs)}
                fillers, ready = attn_fillers(w, pos)
                weave(bigs, fillers, ready)
            # after the last window's attention, all PSUM pools are free:
            # cycle the final proj units across them so the evacuation chain
            # never blocks the PE on a bank WAR
            # final batch: pair adjacent nwi units into one [128,1024] ysb
            # tile - one DMA per pair, copies split across ACT and DVE,
            # pair DMAs alternating both HWDGE queues (exp stream is over)
            def final_pair(tt, nwi0, dmae):
                ysb2 = ypool2.tile([128, 1024], F32, tag="ysb2", name="ysb2")
                for half in range(2):
                    nwi = nwi0 + half
                    # half 0 in psA, half 1 in psS (free after attention):
                    # the next pair's bank is then recycled a full pair
                    # earlier, so its first matmul never waits on the
                    # previous pair's trailing evacuation copy
                    if half == 0:
                        yps = psA.tile([128, 512], F32, tag="ps", name="yps")
                    else:
                        yps = psS.tile([128, 512], F32, tag="st", name="yps")
                    for hh in range(HL):
                        nc.tensor.matmul(
                            yps[:],
                            o_sb[:, hh * T + tt * 128 : hh * T + (tt + 1) * 128],
                            wp_sb[:, hh, nwi * 512 : (nwi + 1) * 512],
                            start=(hh == 0),
                            stop=(hh == HL - 1),
                        )
                    if half == 0:
                        nc.scalar.copy(ysb2[:, 0:512], yps[:])
                    else:
                        nc.vector.tensor_copy(ysb2[:, 512:1024], yps[:])
                dmae.dma_start(
                    y[tt * 128 : (tt + 1) * 128, nwi0 * 512 : (nwi0 + 2) * 512],
                    ysb2[:],
                )

            pi = 0
            for tt in range(4 * (NW - 1), 4 * NW):
                for nwi0 in (0, 2):
                    if tt == 4 * NW - 1 and nwi0 == 2:
                        break
                    final_pair(tt, nwi0, nc.scalar if pi % 2 else nc.sync)
                    pi += 1
            # final unit split into column halves on both HWDGE queues so the
            # exposed tail evacuation + y DMA is halved
            tt = NTT - 1
            r0 = slice(tt * 128, (tt + 1) * 128)
            for nwi, half in ((2, 0), (3, 1)):
                yps = psA.tile([128, 512], F32, tag="ps", name="yps")
                for hh in range(HL):
                    nc.tensor.matmul(
                        yps[:],
                        o_sb[:, hh * T + tt * 128 : hh * T + (tt + 1) * 128],
                        wp_sb[:, hh, nwi * 512 : (nwi + 1) * 512],
                        start=(hh == 0),
                        stop=(hh == HL - 1),
                    )
                ysb = ypool.tile([128, 512], F32, tag="ysb", name="ysb")
                if half == 0:
                    nc.vector.tensor_copy(ysb[:], yps[:])
                    nc.sync.dma_start(
                        y[r0, nwi * 512 : (nwi + 1) * 512], ysb[:]
                    )
                else:
                    nc.scalar.copy(ysb[:, 0:256], yps[:, 0:256])
                    nc.scalar.dma_start(
                        y[r0, nwi * 512 : nwi * 512 + 256], ysb[:, 0:256]
                    )
                    nc.vector.tensor_copy(ysb[:, 256:512], yps[:, 256:512])
                    nc.sync.dma_start(
                        y[r0, nwi * 512 + 256 : (nwi + 1) * 512], ysb[:, 256:512]
                    )

    nc.compile()
    return nc


def _rope_tables():
    inv_freq = (
        1.0 / (10000.0 ** (np.arange(0, D, 2, dtype=np.float32) / np.float32(D)))
    ).astype(np.float32)
    tpos = np.arange(T, dtype=np.float32)
    freqs = tpos[:, None] * inv_freq[None, :]  # (T, 64)
    cosT = np.ascontiguousarray(np.cos(freqs).T)  # (64, T)
    sinT = np.ascontiguousarray(np.sin(freqs).T)
    return (
        cosT.astype(ml_dtypes.bfloat16),
        sinT.astype(ml_dtypes.bfloat16),
    )


def make_in_maps(x, W_qkv, W_proj):
    cosT, sinT = _rope_tables()
    tri = (np.arange(128)[None, :] >= np.arange(128)[:, None]).astype(
        ml_dtypes.bfloat16
    )
    tri = np.ascontiguousarray(tri)
    ones = np.ones((128, 128), dtype=np.float16)

    xPs = {}
    for b in range(B):
        xt = np.ascontiguousarray(x[b].T).astype(ml_dtypes.bfloat16)  # (dim, T)
        xPs[b] = np.ascontiguousarray(
            xt.reshape(NCHUNK, 128, NW, 512).transpose(1, 2, 0, 3)
        )

    wqPs, wvPs, wpPs = {}, {}, {}
    for g in range(4):
        Wq = W_qkv[512 * g : 512 * (g + 1)]
        Wk = W_qkv[2048 + 512 * g : 2048 + 512 * (g + 1)]
        Wv = W_qkv[4096 + 512 * g : 4096 + 512 * (g + 1)]
        Wc = np.concatenate([Wq, Wk, Wv], axis=0)  # (1536, 2048)
        A = (
            np.ascontiguousarray(Wc.T)
            .astype(ml_dtypes.bfloat16)
            .reshape(NCHUNK, 128, 1536)
            .transpose(1, 0, 2)
        )  # [p, c, e]
        wqPs[g] = np.ascontiguousarray(
            A[:, :, :1024].reshape(128, NCHUNK, 8, 128).transpose(0, 2, 1, 3)
        )  # [p, s, c, e]
        wvPs[g] = np.ascontiguousarray(A[:, :, 1024:])  # [p, c, e512]
        wpPs[g] = np.ascontiguousarray(
            np.ascontiguousarray(W_proj[:, 512 * g : 512 * (g + 1)].T)
            .astype(ml_dtypes.bfloat16)
            .reshape(HL, 128, DIM)
            .transpose(1, 0, 2)
        )  # [p, h, n]

    in_maps = []
    for c in range(NCORES):
        b, g = divmod(c, 4)
        in_maps.append(
            {
                "xP": xPs[b],
                "wqP": wqPs[g],
                "wvP": wvPs[g],
                "wpP": wpPs[g],
                "cosT": cosT,
                "sinT": sinT,
                "tri": tri,
                "ones": ones,
            }
        )
    return in_maps


def kernel(x, W_qkv, W_proj):
    global LAST_RESULTS
    x = np.asarray(x, dtype=np.float32)
    W_qkv = np.asarray(W_qkv, dtype=np.float32)
    W_proj = np.asarray(W_proj, dtype=np.float32)
    assert x.shape == (B, T, DIM) and W_qkv.shape == (3 * H * D, DIM)

    if "nc" not in _CACHE:
        _CACHE["nc"] = _build_module()
    nc = _CACHE["nc"]

    in_maps = make_in_maps(x, W_qkv, W_proj)
    trace = os.environ.get("KERNEL_TRACE", "0") == "1"
    res = bass_utils.run_bass_kernel_spmd(
        nc, in_maps, core_ids=list(range(NCORES)), trace=trace
    )
    LAST_RESULTS = res
    y = np.zeros((B, T, DIM), dtype=np.float32)
    for c in range(NCORES):
        y[c // 4] += res.results[c]["y"]
    return y


# revision 33
# speedup vs baseline: 1.0404x; 1.0164x over previous
"""Causal self-attention (B=2, T=2048, dim=2048, H=16, D=128) on 8 trn2 NeuronCores.

Sharding: data-parallel over batch (2 groups of 4 cores), tensor-parallel over
heads within a group (4 heads/core).  Each core computes its heads' QKV
projection (x @ Wqkv_part^T), RoPE, causal attention, and a partial output
projection against its W_proj column block; the host sums the 4 partials per
batch element.

v8 schedule (all matmul operands bf16, fp32 accumulation): one readiness-aware
software pipeline — attention units of window w are woven between the QKV
matmul sets of the SAME window and the proj units of window w-1, so the
ScalarE exp stream always drains under QKV/proj PE work:
  - all DRAM operands host-packed so every DMA lands in >=4KB contiguous
    per-partition lines; startup loads split across the two HWDGE queues
    (sync + scalar) in first-use order; first weight slab split so the
    first matmul gates on 128KB; rope tables shipped as 64 rows and
    mirrored on-chip (rows 0-63 == rows 64-127 up to the rotate sign).
  - RoPE fused into the QKV PSUM evacuation on DVE: rot halves are
    partition-offset multiplies against the sign-folded sin table, then
    dst = ps*cos + rot.  No PE swap matmul, no separate evacuation cast.
  - scores pipelined depth-3: PE score matmul -> ScalarE exp (bf16) -> PE
    PV.  Each head-window's pt tiles chain-accumulate into one fp16 tile
    on DVE (one add per unit), so the PE rowsum is a single 512-col
    all-ones matmul per head (denominators replicated across partitions).
  - softmax normalization off the PE: reciprocal_approx_fast + multiply (DVE).
  - QKV/proj PSUM evacuation on DVE/ScalarE; y DMA'd per window from a
    6-deep tile pool so evacuations never block on the y-queue drain; the
    final window's proj pairs two 512-col units per y DMA, splits their
    evacuations across ACT and DVE, alternates both HWDGE queues, and
    spreads PSUM across the attention pools (free by then).
"""

import os

import numpy as np
import ml_dtypes

import concourse.bass as bass
import concourse.bacc as bacc
import concourse.tile as tile
import concourse.mybir as mybir
from concourse import bass_utils

BF16 = mybir.dt.bfloat16
F16 = mybir.dt.float16
F32 = mybir.dt.float32

B, T, DIM = 2, 2048, 2048
H, D = 16, 128
HL = 4                   # heads per core
NCORES = 8
NCHUNK = DIM // 128      # 16 contraction chunks
NW = T // 512            # 4 query windows
NTT = T // 128           # 16 token tiles
SCALE = 1.0 / float(np.sqrt(D))

_CACHE = {}
LAST_RESULTS = None


def _build_module():
    nc = bacc.Bacc("TRN2", target_bir_lowering=False, debug=False)
    # host-packed layouts: partition dim first, contiguous per-partition lines
    xPd = nc.dram_tensor("xP", (128, NW, NCHUNK, 512), BF16, kind="ExternalInput")
    wqPd = nc.dram_tensor("wqP", (128, 8, NCHUNK, 128), BF16, kind="ExternalInput")
    wvPd = nc.dram_tensor("wvP", (128, NCHUNK, 512), BF16, kind="ExternalInput")
    wpPd = nc.dram_tensor("wpP", (128, HL, DIM), BF16, kind="ExternalInput")
    cosT = nc.dram_tensor("cosT", (64, T), BF16, kind="ExternalInput")
    sinT = nc.dram_tensor("sinT", (64, T), BF16, kind="ExternalInput")
    tri = nc.dram_tensor("tri", (128, 128), BF16, kind="ExternalInput")
    ones = nc.dram_tensor("ones", (128, 128), F16, kind="ExternalInput")
    y = nc.dram_tensor("y", (T, DIM), F32, kind="ExternalOutput")

    Exp = mybir.ActivationFunctionType.Exp

    with tile.TileContext(nc) as tc:
        with (
            tc.tile_pool(name="const", bufs=1) as cpool,
            tc.tile_pool(name="xp", bufs=2) as xpool,
            tc.tile_pool(name="rotp", bufs=3) as rotpool,
            tc.tile_pool(name="ptp", bufs=6) as ptpool,
            tc.tile_pool(name="rsap", bufs=2) as rsapool,
            tc.tile_pool(name="rcpp", bufs=2) as rcppool,
            tc.tile_pool(name="yp", bufs=6) as ypool,
            tc.tile_pool(name="yp2", bufs=3) as ypool2,
            tc.tile_pool(name="psA", bufs=2, space="PSUM") as psA,
            tc.tile_pool(name="psS", bufs=3, space="PSUM") as psS,
            tc.tile_pool(name="psO", bufs=2, space="PSUM") as psO,
            tc.tile_pool(name="psR", bufs=1, space="PSUM") as psR,
        ):
            # per-set weight slabs: dependency granularity is per-tile, so a
            # qk set only waits for its own slab's DMA; slab 0 split so the
            # first matmul gates on a 128KB transfer
            wq0a = cpool.tile([128, 2, 128], BF16, tag="wq0a", name="wq0a")
            wq0b = cpool.tile([128, 14, 128], BF16, tag="wq0b", name="wq0b")
            wq_slab = [None] + [
                cpool.tile([128, NCHUNK, 128], BF16, tag=f"wq{s}", name=f"wq{s}")
                for s in range(1, 8)
            ]
            wv_sb = cpool.tile([128, NCHUNK, 512], BF16, tag="wv")
            wp_sb = cpool.tile([128, HL, DIM], BF16, tag="wp")
            cos_sb = cpool.tile([128, T], BF16, tag="cos")
            sin_sb = cpool.tile([128, T], BF16, tag="sin")
            tri_sb = cpool.tile([128, 128], BF16, tag="tri")
            ones_sb = cpool.tile([128, 128], F16, tag="ones")
            q_sb = cpool.tile([128, HL * T], BF16, tag="q")
            k_sb = cpool.tile([128, HL * T], BF16, tag="k")
            v_sb = cpool.tile([128, NTT * HL * D], BF16, tag="v")
            o_sb = cpool.tile([128, HL * T], BF16, tag="o")

            xbs = {}

            def load_xb(w):
                xb = xpool.tile([128, NCHUNK, 512], BF16, tag="xb", name="xb")
                nc.sync.dma_start(xb[:], xPd[:, w])
                xbs[w] = xb

            # Startup DMAs, first-needed first, split across both HWDGE
            # queues (sync + scalar); xb0 in 2-chunk pieces alternating
            # queues so set-0's chunk loop rarely outruns arrival; half-height
            # rope tables on the gpsimd SWDGE, mirrored on-chip by DVE.
            xb0 = xpool.tile([128, NCHUNK, 512], BF16, tag="xb", name="xb0")
            xbs[0] = xb0
            nc.scalar.dma_start(wq0a[:], wqPd[:, 0, 0:2, :])
            nc.sync.dma_start(xb0[:, 0:1, :], xPd[:, 0, 0:1, :])
            nc.gpsimd.dma_start(cos_sb[0:64, :], cosT[:, :])
            nc.gpsimd.dma_start(sin_sb[64:128, :], sinT[:, :])
            nc.scalar.dma_start(wq0b[:], wqPd[:, 0, 2:16, :])
            nc.sync.dma_start(xb0[:, 1:3, :], xPd[:, 0, 1:3, :])
            nc.sync.dma_start(xb0[:, 3:6, :], xPd[:, 0, 3:6, :])
            nc.scalar.dma_start(xb0[:, 6:8, :], xPd[:, 0, 6:8, :])
            nc.sync.dma_start(xb0[:, 8:10, :], xPd[:, 0, 8:10, :])
            nc.scalar.dma_start(wq_slab[1][:], wqPd[:, 1])
            nc.sync.dma_start(xb0[:, 10:12, :], xPd[:, 0, 10:12, :])
            nc.scalar.dma_start(xb0[:, 12:14, :], xPd[:, 0, 12:14, :])
            nc.sync.dma_start(xb0[:, 14:16, :], xPd[:, 0, 14:16, :])
            nc.gpsimd.dma_start(tri_sb[:], tri[:, :])
            nc.gpsimd.dma_start(ones_sb[:], ones[:, :])
            nc.scalar.dma_start(wq_slab[2][:], wqPd[:, 2])
            nc.sync.dma_start(wq_slab[3][:], wqPd[:, 3])
            nc.sync.dma_start(wv_sb[:, 0:8, :], wvPd[:, 0:8, :])
            nc.scalar.dma_start(wv_sb[:, 8:16, :], wvPd[:, 8:16, :])
            nc.sync.dma_start(wq_slab[4][:], wqPd[:, 4])
            nc.scalar.dma_start(wq_slab[5][:], wqPd[:, 5])
            nc.sync.dma_start(wq_slab[6][:], wqPd[:, 6])
            nc.scalar.dma_start(wq_slab[7][:], wqPd[:, 7])
            # mirror the half-height tables on DVE (cos rows repeat; sin rows
            # 0-63 carry the folded rotate-half sign)
            nc.vector.tensor_copy(cos_sb[64:128, :], cos_sb[0:64, :])
            nc.vector.tensor_scalar_mul(sin_sb[0:64, :], sin_sb[64:128, :], -1.0)

            def qkv_bigs(w):
                """[('q'|'k'|'v', idx, closure)] — 8 qk sets then 4 v sets."""
                wsl = slice(w * 512, (w + 1) * 512)

                def qk_set(grp, j):
                    def run():
                        xb = xbs[w]
                        dst = q_sb if grp == 0 else k_sb
                        ps = psA.tile([128, 512], F32, tag="ps", name="ps")
                        s = grp * 4 + j
                        for c in range(NCHUNK):
                            if s == 0:
                                slab = wq0a[:, c, :] if c < 2 else wq0b[:, c - 2, :]
                            else:
                                slab = wq_slab[s][:, c, :]
                            nc.tensor.matmul(
                                ps[:],
                                slab,
                                xb[:, c, :],
                                start=(c == 0),
                                stop=(c == NCHUNK - 1),
                            )
                        # fused RoPE evacuation: dst = ps*cos + rot(ps)*sin
                        sl = slice(j * T + w * 512, j * T + (w + 1) * 512)
                        rot = rotpool.tile([128, 512], BF16, tag="rot", name="rot")
                        nc.vector.tensor_mul(
                            rot[0:64, :], ps[64:128, :], sin_sb[0:64, wsl]
                        )
                        nc.vector.tensor_mul(
                            rot[64:128, :], ps[0:64, :], sin_sb[64:128, wsl]
                        )
                        nc.vector.tensor_mul(dst[:, sl], ps[:], cos_sb[:, wsl])
                        nc.vector.tensor_add(dst[:, sl], dst[:, sl], rot[:])

                    return run

                def v_set(ttl):
                    def run():
                        xb = xbs[w]
                        ttg = w * 4 + ttl
                        ps = psA.tile([128, 512], F32, tag="ps", name="ps")
                        for c in range(NCHUNK):
                            nc.tensor.matmul(
                                ps[:],
                                xb[:, c, ttl * 128 : (ttl + 1) * 128],
                                wv_sb[:, c, :],
                                start=(c == 0),
                                stop=(c == NCHUNK - 1),
                            )
                        nc.scalar.copy(v_sb[:, ttg * 512 : (ttg + 1) * 512], ps[:])

                    return run

                return (
                    [("q", j, qk_set(0, j)) for j in range(HL)]
                    + [("v", ttl, v_set(ttl)) for ttl in range(4)]
                    + [("k", j, qk_set(1, j)) for j in range(HL)]
                )

            def attn_fillers(w, pos):
                """Per-(head, key-tile) closures + their readiness (number of
                this window's qkv bigs that must have been emitted first).
                All of a head-window's pt tiles chain-accumulate into one
                fp16 acc on DVE (one add per unit); a single 512-col
                all-ones matmul per head produces the softmax denominators,
                emitted one unit into the next head so the single-bank
                rowsum tile's WAR on the previous reciprocal is covered."""
                nkt = 4 * w + 4
                OD = 4 * w            # off-diagonal tiles per head
                nu = HL * nkt
                state = {
                    "pend": [],
                    "issued": 0,
                    "oT": {},
                    "pt0": None,
                    "acc": {},
                }

                def geom(kt):
                    if kt < OD:
                        return 512 * w, 512, False
                    q0 = 128 * kt
                    return q0, 512 * (w + 1) - 128 * kt, True

                def issue_score(u):
                    h, kt = divmod(u, nkt)
                    hq = h * T
                    q0, n, diag = geom(kt)
                    st = psS.tile([128, 512], F32, tag="st", name="st")
                    nc.tensor.matmul(
                        st[:, :n],
                        k_sb[:, hq + kt * 128 : hq + (kt + 1) * 128],
                        q_sb[:, hq + q0 : hq + q0 + n],
                        start=True,
                        stop=True,
                    )
                    pt = ptpool.tile([128, 512], BF16, tag="pt", name="pt")
                    nc.scalar.activation(
                        pt[:, :n], st[:, :n], Exp, bias=0.0, scale=SCALE
                    )
                    if diag:
                        nc.vector.tensor_mul(pt[:, 0:128], pt[:, 0:128], tri_sb[:])
                    return pt, q0, n

                DEPTH = 3

                def flush_rs(h):
                    # single rowsum matmul for head h, then the off-PE
                    # normalization chain
                    rs = psR.tile([128, 512], F32, tag="rs", name="rs")
                    nc.tensor.matmul(
                        rs[:],
                        ones_sb[:],
                        state["acc"].pop(h)[:],
                        start=True,
                        stop=True,
                    )
                    hq = h * T
                    rcp = rcppool.tile([128, 512], F32, tag="rcp", name="rcp")
                    nc.vector.reciprocal_approx_fast(rcp[:], rs[:])
                    nc.vector.tensor_mul(
                        o_sb[:, hq + w * 512 : hq + (w + 1) * 512],
                        state["oT"].pop(h)[:],
                        rcp[:],
                    )

                def unit(u):
                    def run():
                        h, kt = divmod(u, nkt)
                        while state["issued"] < min(u + DEPTH + 1, nu):
                            state["pend"].append(issue_score(state["issued"]))
                            state["issued"] += 1
                        if kt == 0:
                            state["oT"][h] = psO.tile(
                                [128, 512], F32, tag="oT", name="oT"
                            )
                        pt, q0, n = state["pend"].pop(0)
                        off = q0 - 512 * w
                        nc.tensor.matmul(
                            state["oT"][h][:, off:512],
                            v_sb[:, kt * 512 + h * 128 : kt * 512 + (h + 1) * 128],
                            pt[:, :n],
                            start=(kt == 0),
                            stop=(kt == nkt - 1),
                        )
                        # denominator accumulation (kt = 0 always has n = 512)
                        if kt == 0:
                            state["pt0"] = pt
                        elif kt == 1:
                            acc = rsapool.tile(
                                [128, 512], F16, tag="acc", name="acc"
                            )
                            if n == 512:
                                nc.vector.tensor_add(
                                    acc[:], state["pt0"][:], pt[:]
                                )
                            else:
                                nc.vector.tensor_copy(
                                    acc[:, 0:off], state["pt0"][:, 0:off]
                                )
                                nc.vector.tensor_add(
                                    acc[:, off:512],
                                    state["pt0"][:, off:512],
                                    pt[:, :n],
                                )
                            state["acc"][h] = acc
                        else:
                            acc = state["acc"][h]
                            nc.vector.tensor_add(
                                acc[:, off:512], acc[:, off:512], pt[:, :n]
                            )
                        if kt == 1 and h > 0:
                            flush_rs(h - 1)
                        if u == nu - 1:
                            flush_rs(h)

                    return run

                def score_ra(u):
                    h, kt = divmod(u, nkt)
                    if kt < OD:
                        return pos[("q", h)] + 1
                    return pos[("k", h)] + 1

                def pv_ra(u):
                    h, kt = divmod(u, nkt)
                    if kt < OD:
                        return 0
                    return pos[("v", kt - OD)] + 1

                fillers = []
                ready = []
                for u in range(nu):
                    ra = pv_ra(u)
                    for uu in range(u, min(u + DEPTH + 1, nu)):
                        ra = max(ra, score_ra(uu))
                    fillers.append(unit(u))
                    ready.append(ra)
                return fillers, ready

            def proj_bigs(w, pools=None, alt_dma=False):
                out = []

                def unit(tt, nwi, pool, ptag, dve_evac, dmae):
                    def run():
                        yps = pool.tile([128, 512], F32, tag=ptag, name="yps")
                        for hh in range(HL):
                            nc.tensor.matmul(
                                yps[:],
                                o_sb[:, hh * T + tt * 128 : hh * T + (tt + 1) * 128],
                                wp_sb[:, hh, nwi * 512 : (nwi + 1) * 512],
                                start=(hh == 0),
                                stop=(hh == HL - 1),
                            )
                        ysb = ypool.tile([128, 512], F32, tag="ysb", name="ysb")
                        if dve_evac:
                            nc.vector.tensor_copy(ysb[:], yps[:])
                        else:
                            nc.scalar.copy(ysb[:], yps[:])
                        dmae.dma_start(
                            y[tt * 128 : (tt + 1) * 128, nwi * 512 : (nwi + 1) * 512],
                            ysb[:],
                        )

                    return run

                i = 0
                for tt in range(4 * w, 4 * w + 4):
                    for nwi in range(DIM // 512):
                        if pools:
                            pool, ptag = pools[i % len(pools)]
                        else:
                            pool, ptag = psA, "ps"
                        # in the final batch (attention over) split the
                        # evacuations across ACT and DVE and put each unit's
                        # DMA config on the opposite sequencer so neither
                        # sequencer serializes copy + trigger
                        dve = alt_dma and (i % 2 == 1)
                        dmae = nc.scalar if dve else nc.sync
                        out.append(
                            ("p", i, unit(tt, nwi, pool, ptag, dve, dmae))
                        )
                        i += 1
                return out

            def weave(bigs, fillers, ready):
                nb = len(bigs)
                nf = len(fillers)
                done = 0
                for i, (_, _, b) in enumerate(bigs):
                    b()
                    # cap the per-big burst so late-ready units don't flood
                    # the exp engine all at once
                    target = min(int(round(nf * (i + 1) / nb)), done + 4)
                    while done < nf and done < target and ready[done] <= i + 1:
                        fillers[done]()
                        done += 1
                while done < nf:
                    fillers[done]()
                    done += 1

            for w in range(NW):
                bigs = qkv_bigs(w)
                if w + 1 < NW:
                    # prefetch next window's x once the startup crunch is over
                    bigs.insert(6, ("x", w + 1, lambda wn=w + 1: load_xb(wn)))
                if w == 0:
                    bigs.append(
                        ("wp", 0, lambda: nc.sync.dma_start(wp_sb[:], wpPd[:]))
                    )
                if w >= 1:
                    bigs += proj_bigs(w - 1)
                pos = {(lbl, idx): i for i, (lbl, idx, _) in enumerate(bigs)}
                fillers, ready = attn_fillers(w, pos)
                weave(bigs, fillers, ready)
            # after the last window's attention, all PSUM pools are free:
            # cycle the final proj units across them so the evacuation chain
            # never blocks the PE on a bank WAR
            # final batch: pair adjacent nwi units into one [128,1024] ysb
            # tile - one DMA per pair, copies split across ACT and DVE,
            # pair DMAs alternating both HWDGE queues (exp stream is over)
            def final_pair(tt, nwi0, dmae):
                ysb2 = ypool2.tile([128, 1024], F32, tag="ysb2", name="ysb2")
                for half in range(2):
                    nwi = nwi0 + half
                    # half 0 in psA, half 1 in psS (free after attention):
                    # the next pair's bank is then recycled a full pair
                    # earlier, so its first matmul never waits on the
                    # previous pair's trailing evacuation copy
                    if half == 0:
                        yps = psA.tile([128, 512], F32, tag="ps", name="yps")
                    else:
                        yps = psS.tile([128, 512], F32, tag="st", name="yps")
                    for hh in range(HL):
                        nc.tensor.matmul(
                            yps[:],
                            o_sb[:, hh * T + tt * 128 : hh * T + (tt + 1) * 128],
                            wp_sb[:, hh, nwi * 512 : (nwi + 1) * 512],
                            start=(hh == 0),
                            stop=(hh == HL - 1),
                        )
                    if half == 0:
                        nc.scalar.copy(ysb2[:, 0:512], yps[:])
                    else:
                        nc.vector.tensor_copy(ysb2[:, 512:1024], yps[:])
                dmae.dma_start(
                    y[tt * 128 : (tt + 1) * 128, nwi0 * 512 : (nwi0 + 2) * 512],
                    ysb2[:],
                )

            pi = 0
            for tt in range(4 * (NW - 1), 4 * NW):
                for nwi0 in (0, 2):
                    if tt == 4 * NW - 1 and nwi0 == 2:
                        break
                    final_pair(tt, nwi0, nc.scalar if pi % 2 else nc.sync)
                    pi += 1
            # final unit split into column halves on both HWDGE queues so the
            # exposed tail evacuation + y DMA is halved
            tt = NTT - 1
            r0 = slice(tt * 128, (tt + 1) * 128)
            for nwi, half in ((2, 0), (3, 1)):
                yps = psA.tile([128, 512], F32, tag="ps", name="yps")
                for hh in range(HL):
                    nc.tensor.matmul(
                        yps[:],
                        o_sb[:, hh * T + tt * 128 : hh * T + (tt + 1) * 128],
                        wp_sb[:, hh, nwi * 512 : (nwi + 1) * 512],
                        start=(hh == 0),
                        stop=(hh == HL - 1),
                    )
                ysb = ypool.tile([128, 512], F32, tag="ysb", name="ysb")
                if half == 0:
                    nc.vector.tensor_copy(ysb[:], yps[:])
                    nc.sync.dma_start(
                        y[r0, nwi * 512 : (nwi + 1) * 512], ysb[:]
                    )
                else:
                    nc.scalar.copy(ysb[:, 0:256], yps[:, 0:256])
                    nc.scalar.dma_start(
                        y[r0, nwi * 512 : nwi * 512 + 256], ysb[:, 0:256]
                    )
                    nc.vector.tensor_copy(ysb[:, 256:512], yps[:, 256:512])
                    nc.sync.dma_start(
                        y[r0, nwi * 512 + 256 : (nwi + 1) * 512], ysb[:, 256:512]
                    )

    nc.compile()
    return nc


def _rope_tables():
    inv_freq = (
        1.0 / (10000.0 ** (np.arange(0, D, 2, dtype=np.float32) / np.float32(D)))
    ).astype(np.float32)
    tpos = np.arange(T, dtype=np.float32)
    freqs = tpos[:, None] * inv_freq[None, :]  # (T, 64)
    cosT = np.ascontiguousarray(np.cos(freqs).T)  # (64, T)
    sinT = np.ascontiguousarray(np.sin(freqs).T)
    return (
        cosT.astype(ml_dtypes.bfloat16),
        sinT.astype(ml_dtypes.bfloat16),
    )


def make_in_maps(x, W_qkv, W_proj):
    cosT, sinT = _rope_tables()
    tri = (np.arange(128)[None, :] >= np.arange(128)[:, None]).astype(
        ml_dtypes.bfloat16
    )
    tri = np.ascontiguousarray(tri)
    ones = np.ones((128, 128), dtype=np.float16)

    xPs = {}
    for b in range(B):
        xt = np.ascontiguousarray(x[b].T).astype(ml_dtypes.bfloat16)  # (dim, T)
        xPs[b] = np.ascontiguousarray(
            xt.reshape(NCHUNK, 128, NW, 512).transpose(1, 2, 0, 3)
        )

    wqPs, wvPs, wpPs = {}, {}, {}
    for g in range(4):
        Wq = W_qkv[512 * g : 512 * (g + 1)]
        Wk = W_qkv[2048 + 512 * g : 2048 + 512 * (g + 1)]
        Wv = W_qkv[4096 + 512 * g : 4096 + 512 * (g + 1)]
        Wc = np.concatenate([Wq, Wk, Wv], axis=0)  # (1536, 2048)
        A = (
            np.ascontiguousarray(Wc.T)
            .astype(ml_dtypes.bfloat16)
            .reshape(NCHUNK, 128, 1536)
            .transpose(1, 0, 2)
        )  # [p, c, e]
        wqPs[g] = np.ascontiguousarray(
            A[:, :, :1024].reshape(128, NCHUNK, 8, 128).transpose(0, 2, 1, 3)
        )  # [p, s, c, e]
        wvPs[g] = np.ascontiguousarray(A[:, :, 1024:])  # [p, c, e512]
        wpPs[g] = np.ascontiguousarray(
            np.ascontiguousarray(W_proj[:, 512 * g : 512 * (g + 1)].T)
            .astype(ml_dtypes.bfloat16)
            .reshape(HL, 128, DIM)
            .transpose(1, 0, 2)
        )  # [p, h, n]

    in_maps = []
    for c in range(NCORES):
        b, g = divmod(c, 4)
        in_maps.append(
            {
                "xP": xPs[b],
                "wqP": wqPs[g],
                "wvP": wvPs[g],
                "wpP": wpPs[g],
                "cosT": cosT,
                "sinT": sinT,
                "tri": tri,
                "ones": ones,
            }
        )
    return in_maps


def kernel(x, W_qkv, W_proj):
    global LAST_RESULTS
    x = np.asarray(x, dtype=np.float32)
    W_qkv = np.asarray(W_qkv, dtype=np.float32)
    W_proj = np.asarray(W_proj, dtype=np.float32)
    assert x.shape == (B, T, DIM) and W_qkv.shape == (3 * H * D, DIM)

    if "nc" not in _CACHE:
        _CACHE["nc"] = _build_module()
    nc = _CACHE["nc"]

    in_maps = make_in_maps(x, W_qkv, W_proj)
    trace = os.environ.get("KERNEL_TRACE", "0") == "1"
    res = bass_utils.run_bass_kernel_spmd(
        nc, in_maps, core_ids=list(range(NCORES)), trace=trace
    )
    LAST_RESULTS = res
    y = np.zeros((B, T, DIM), dtype=np.float32)
    for c in range(NCORES):
        y[c // 4] += res.results[c]["y"]
    return y
